# revision 3
# baseline (speedup 1.0000x reference)
"""Trainium2 Bass kernel for nn_CrossmodalFusion (B=1024, R=36, D=1024).

Data-parallel over batch across 8 NeuronCores with token-level sparsity:
the sigmoid attention mask zeroes every region token with j >=
region_lens[b]; for those tokens the output is exactly relu(f1_b). The
host compacts each core's valid tokens (in (batch, region) order), the
device processes only those (~51%), and the host scatters results back.

The four big per-token GEMMs (mi_W1, mi_W2, sc_W, f1_W) run in fp8-e4m3
DoubleRow mode (2 fp8 contraction planes per PE pass -> half the matmul
instructions of bf16). Weights are host-prescaled by a power of two into
fp8 range; the descale folds into the post-matmul activation `scale`.
All intermediate tensors stay bf16; fp8 copies exist only as matmul
inputs, so quantization error does not chain (CPU-model rel err 1.16e-2
vs the 2e-2 gate). The seg/qw path stays bf16 end-to-end.

Attention weights: masked logits are reduced AND broadcast to 128
partitions in a single ones-matmul on the PE (no DRAM bounce). Initial
loads are spread across the sync/scalar/vector/gpsimd DMA rings so tile
0's mm1 starts as soon as x+W1 land. The output is stored bf16.

Because each core's token plan differs, 8 per-core programs are compiled
(concurrently) and dispatched asynchronously, one per NeuronCore.
"""
import os
import sys
import types
from concurrent.futures import ThreadPoolExecutor
from contextlib import ExitStack

sys.path.insert(0, "/opt/trn_rl_repo")

import numpy as np
import ml_dtypes

import concourse.bass as bass
import concourse.tile as tile
from concourse import bacc, mybir
from concourse.masks import make_identity

F32 = mybir.dt.float32
BF16 = mybir.dt.bfloat16
FP8 = mybir.dt.float8e4

NPBF16 = ml_dtypes.bfloat16
NPFP8 = ml_dtypes.float8_e4m3  # TRN e4m3: max normal 240

B, R, D = 1024, 36, 1024
H = D // 2
SEG_C = 133
NCORES = 8
BC = B // NCORES            # batches per core
KC = D // 128               # 8 feature chunks
KH = H // 128               # 4 hidden chunks

TOKCAP = 512                # tokens per tile (PSUM fp32 bank width)
RSQD = float(1.0 / np.sqrt(D))

LAST_EXEC_NS = None
_LAST_TRACE = None


def _wire_ntff_hook():
    if "antenv.axon_hooks" in sys.modules:
        return
    try:
        import trn_agent_boot.trn_boot as tb
        hook = tb._ntff_profile_via_ctypes("/opt/axon/libaxon_pjrt.so")
    except Exception:
        hook = None
    mod = types.ModuleType("antenv.axon_hooks")
    _h = [hook]
    mod.set_axon_ntff_profile_hook = lambda h: _h.__setitem__(0, h)
    mod.get_axon_ntff_profile_hook = lambda: _h[0]
    sys.modules["antenv.axon_hooks"] = mod


def _make_plan(lens_c):
    """Tile plan for one core from its per-batch valid-token counts."""
    stream = []
    for lb, ln in enumerate(lens_c):
        stream.extend((lb, j) for j in range(int(ln)))
    ntokc = len(stream)
    tiles = []
    t0 = 0
    while t0 < ntokc:
        nt = 0
        b_first = stream[t0][0]
        while t0 + nt < ntokc and nt < TOKCAP:
            lb = stream[t0 + nt][0]
            if lb - b_first + 1 > 128:
                break
            nt += 1
        b_last = stream[t0 + nt - 1][0]
        segs = []
        pos = 0
        while pos < nt:
            lb = stream[t0 + pos][0]
            end = pos
            while end < nt and stream[t0 + end][0] == lb:
                end += 1
            segs.append((lb - b_first, pos, end))
            pos = end
        tiles.append(dict(t0=t0, nt=nt, b0=b_first, nb=b_last - b_first + 1, segs=segs))
        t0 += nt
    return tiles, ntokc


def _emit(ctx, tc, plan, scales):
    nc = tc.nc
    AF = mybir.ActivationFunctionType
    ALU = mybir.AluOpType
    DR = mybir.MatmulPerfMode.DoubleRow
    tiles, ntokc = plan
    s1, s2, s3, s4 = scales

    # ---- DRAM I/O -------------------------------------------------------
    x8T = nc.dram_tensor("x8T", [D, ntokc], FP8, kind="ExternalInput").ap()
    xbT = nc.dram_tensor("xbT", [D, ntokc], BF16, kind="ExternalInput").ap()
    unet = nc.dram_tensor("unet", [BC, SEG_C, 49], BF16, kind="ExternalInput").ap()
    ind_sz = sum(t["nb"] * t["nt"] for t in tiles)
    ind_blob = nc.dram_tensor("ind", [ind_sz], BF16, kind="ExternalInput").ap()
    wi = {}
    for name, shape, dt in [
        ("mi_W1q", [D, H], FP8), ("mi_b1", [1, H], F32),
        ("mi_W2q", [H, D], FP8), ("mi_b2", [1, D], F32),
        ("ms_W1", [D, H], BF16), ("ms_b1", [1, H], F32),
        ("ms_W2", [H, D], BF16), ("ms_b2", [1, D], F32),
        ("seg_W", [SEG_C, D], BF16), ("seg_b", [1, D], F32),
        ("ln_g", [1, D], F32), ("ln_b", [1, D], F32),
        ("sc_Wq", [D, D], FP8), ("sc_b", [1, D], F32),
        ("f1_Wq", [D, D], FP8), ("f1_W", [D, D], BF16), ("f1_b", [1, D], F32),
    ]:
        wi[name] = nc.dram_tensor(name, shape, dt, kind="ExternalInput").ap()
    outT = nc.dram_tensor("outT", [D, ntokc], BF16, kind="ExternalOutput").ap()
    fillv = nc.dram_tensor("fillv", [1, D], F32, kind="ExternalOutput").ap()

    qw_scr = nc.dram_tensor("qw_scr", [BC, D], BF16).ap()

    # ---- persistent constants ------------------------------------------
    const = ctx.enter_context(tc.tile_pool(name="const", bufs=1))

    def load_w(eng, name, kchunks, m, dt):
        t = const.tile([128, kchunks, m], dt, tag=f"cw_{name}")
        eng.dma_start(t[:], wi[name].rearrange("(kc p) m -> p kc m", p=128))
        return t

    # sync ring: what mm1/mm2 need first (then x tiles from the main loop)
    W1q = load_w(nc.sync, "mi_W1q", KC, H, FP8)
    W2q = load_w(nc.sync, "mi_W2q", KH, D, FP8)
    # scalar ring: q-stage input first, then later-needed big weights
    W3q = load_w(nc.scalar, "sc_Wq", KC, D, FP8)
    W4q = load_w(nc.scalar, "f1_Wq", KC, D, FP8)

    def load_col(name, mchunks):
        ap_ = wi[name]
        t = const.tile([128, mchunks], F32, tag=f"cc_{name}")
        src = bass.AP(tensor=ap_.tensor, offset=ap_.offset, ap=[[1, 128], [128, mchunks]])
        nc.gpsimd.dma_start(t[:], src)
        return t

    b_mi1c = load_col("mi_b1", KH)
    b_mi2c = load_col("mi_b2", KC)
    b_scc = load_col("sc_b", KC)
    b_f1c = load_col("f1_b", KC)

    ones_row = const.tile([1, 512], BF16)
    nc.vector.memset(ones_row[:], 1.0)
    ones_bc = const.tile([128, 128], BF16)   # lhsT for reduce+broadcast
    nc.vector.memset(ones_bc[:], 1.0)
    ident_bf = const.tile([128, 128], BF16)
    make_identity(nc, ident_bf)
    eps_t = const.tile([128, 1], F32)
    nc.vector.memset(eps_t[:], 1e-5)

    qT_bf = const.tile([128, KC, BC], BF16)    # feature-major q (lhsT for attn)

    psum = ctx.enter_context(tc.tile_pool(name="psum", bufs=1, space="PSUM"))

    # ============================ q-stage ================================
    with tc.tile_pool(name="qpool", bufs=1) as qp:
        unet_sb = qp.tile([BC, SEG_C, 49], BF16)
        nc.scalar.dma_start(unet_sb[:], unet[:, :, :])
        W_seg_a = qp.tile([128, D], BF16)
        nc.scalar.dma_start(W_seg_a[:], wi["seg_W"][0:128, :])
        W_seg_b = qp.tile([5, D], BF16)
        nc.scalar.dma_start(W_seg_b[:], wi["seg_W"][128:SEG_C, :])
        W_ms1 = qp.tile([128, KC, H], BF16)
        nc.gpsimd.dma_start(W_ms1[:], wi["ms_W1"].rearrange("(kc p) m -> p kc m", p=128))
        W_ms2 = qp.tile([128, KH, D], BF16)
        nc.gpsimd.dma_start(W_ms2[:], wi["ms_W2"].rearrange("(kc p) m -> p kc m", p=128))
        W_f1b = qp.tile([128, KC, D], BF16)
        nc.scalar.dma_start(W_f1b[:], wi["f1_W"].rearrange("(kc p) m -> p kc m", p=128))
        b_segr = qp.tile([1, D], BF16)
        nc.gpsimd.dma_start(b_segr[:], wi["seg_b"])
        b_ms1r = qp.tile([1, H], BF16)
        nc.gpsimd.dma_start(b_ms1r[:], wi["ms_b1"])
        b_ms2r = qp.tile([1, D], BF16)
        nc.gpsimd.dma_start(b_ms2r[:], wi["ms_b2"])
        g_bc = qp.tile([128, D], F32)
        nc.gpsimd.dma_start(g_bc[:], bass.AP(tensor=wi["ln_g"].tensor, offset=wi["ln_g"].offset, ap=[[0, 128], [1, D]]))
        bb_bc = qp.tile([128, D], F32)
        nc.gpsimd.dma_start(bb_bc[:], bass.AP(tensor=wi["ln_b"].tensor, offset=wi["ln_b"].offset, ap=[[0, 128], [1, D]]))

        # avgpool(7x7): reduce, scale, PE-transpose
        pooled = qp.tile([BC, SEG_C], F32)
        nc.vector.reduce_sum(pooled[:], unet_sb[:], axis=mybir.AxisListType.X)
        pooled_bf = qp.tile([BC, SEG_C], BF16)
        nc.scalar.mul(pooled_bf[:], pooled[:], 1.0 / 49.0)
        pa_ps = psum.tile([128, BC], BF16, tag="tps", bufs=1)
        nc.tensor.transpose(pa_ps[:], pooled_bf[:, 0:128], ident_bf[0:BC, 0:BC])
        pa_bf = qp.tile([128, BC], BF16)
        nc.scalar.copy(pa_bf[:], pa_ps[:])
        pb_ps = psum.tile([5, BC], BF16, tag="tps", bufs=1)
        nc.tensor.transpose(pb_ps[:], pooled_bf[:, 128:SEG_C], ident_bf[0:BC, 0:BC])
        pb_bf = qp.tile([5, BC], BF16)
        nc.scalar.copy(pb_bf[:], pb_ps[:])

        # q1 = relu(pooled @ seg_W + seg_b)   (token-major: BC x D)
        q1 = qp.tile([BC, D], F32)
        for ng in range(2):
            sl = slice(ng * 512, (ng + 1) * 512)
            ps = psum.tile([BC, 512], F32, tag="mmps", bufs=4)
            nc.tensor.matmul(ps[:], pa_bf[:], W_seg_a[:, sl], start=True, stop=False)
            nc.tensor.matmul(ps[:], pb_bf[:], W_seg_b[:, sl], start=False, stop=False)
            nc.tensor.matmul(ps[:], ones_row[0:1, 0:BC], b_segr[0:1, sl], start=False, stop=True)
            nc.vector.tensor_scalar_max(q1[:, sl], ps[:], 0.0)

        # layernorm over D
        stats = qp.tile([BC, 2, 6], F32)
        for s in range(2):
            nc.vector.bn_stats(stats[:, s, :], q1[:, s * 512:(s + 1) * 512])
        mv = qp.tile([BC, 2], F32)
        nc.vector.bn_aggr(mv[:], stats[:])
        rstd = qp.tile([BC, 1], F32)
        nc.scalar.activation(rstd[:], mv[:, 1:2], AF.Sqrt, bias=eps_t[0:BC, :])
        nc.vector.reciprocal(rstd[:], rstd[:])
        qn = qp.tile([BC, D], F32)
        nc.vector.tensor_scalar(qn[:], q1[:], mv[:, 0:1], rstd[:],
                                op0=ALU.subtract, op1=ALU.mult)
        nc.vector.tensor_mul(qn[:], qn[:], g_bc[0:BC, :])
        qn_bf = qp.tile([BC, D], BF16)
        nc.vector.tensor_add(qn_bf[:], qn[:], bb_bc[0:BC, :])

        # qnT (feature-major) via PE transposes
        qnT_bf = qp.tile([128, KC, BC], BF16)
        for kc in range(KC):
            pt = psum.tile([128, BC], BF16, tag="tps", bufs=1)
            nc.tensor.transpose(pt[:], qn_bf[:, kc * 128:(kc + 1) * 128], ident_bf[0:BC, 0:BC])
            nc.scalar.copy(qnT_bf[:, kc, :], pt[:])

        # q MLP (feature-major): qm = relu(ms_W1.T @ qnT + b1)
        qmT_bf = qp.tile([128, KH, BC], BF16)
        for mc in range(KH):
            sl = slice(mc * 128, (mc + 1) * 128)
            ps = psum.tile([128, BC], F32, tag="mmps", bufs=4)
            for kc in range(KC):
                nc.tensor.matmul(ps[:], W_ms1[:, kc, sl], qnT_bf[:, kc, :],
                                 start=(kc == 0), stop=False)
            nc.tensor.matmul(ps[:], b_ms1r[0:1, sl], ones_row[0:1, 0:BC],
                             start=False, stop=True)
            nc.scalar.activation(qmT_bf[:, mc, :], ps[:], AF.Relu)
        # q2T = ms_W2.T @ qmT + b2 + qnT   -> qT_bf
        for mc in range(KC):
            sl = slice(mc * 128, (mc + 1) * 128)
            ps = psum.tile([128, BC], F32, tag="mmps", bufs=4)
            for kc in range(KH):
                nc.tensor.matmul(ps[:], W_ms2[:, kc, sl], qmT_bf[:, kc, :],
                                 start=(kc == 0), stop=False)
            nc.tensor.matmul(ps[:], b_ms2r[0:1, sl], ones_row[0:1, 0:BC],
                             start=False, stop=True)
            nc.vector.tensor_add(qT_bf[:, mc, :], ps[:], qnT_bf[:, mc, :])

        # qw = s4 * (q2 @ f1_W)  (token-major bf16, prescaled to match fp8 psum)
        qw_bf = qp.tile([BC, D], BF16)
        for ng in range(2):
            sl = slice(ng * 512, (ng + 1) * 512)
            ps = psum.tile([BC, 512], F32, tag="mmps", bufs=4)
            for kc in range(KC):
                nc.tensor.matmul(ps[:], qT_bf[:, kc, :], W_f1b[:, kc, sl],
                                 start=(kc == 0), stop=(kc == KC - 1))
            nc.scalar.activation(qw_bf[:, sl], ps[:], AF.Identity, scale=float(s4))
        nc.sync.dma_start(qw_scr[:, :], qw_bf[:])

        # fill vector for masked tokens: relu(f1_b)
        fb_row = qp.tile([1, D], F32)
        nc.gpsimd.dma_start(fb_row[:], wi["f1_b"])
        fb_out = qp.tile([1, D], F32)
        nc.vector.tensor_scalar_max(fb_out[:], fb_row[:], 0.0)
        nc.sync.dma_start(fillv[:, :], fb_out[:])

    # ============================ main loop ==============================
    xp8 = ctx.enter_context(tc.tile_pool(name="xp8", bufs=2))
    xpb = ctx.enter_context(tc.tile_pool(name="xpb", bufs=2))
    hp = ctx.enter_context(tc.tile_pool(name="hp", bufs=2))
    rp = ctx.enter_context(tc.tile_pool(name="rp", bufs=2))
    wcp = ctx.enter_context(tc.tile_pool(name="wcp", bufs=2))
    wc8p = ctx.enter_context(tc.tile_pool(name="wc8p", bufs=2))
    scp = ctx.enter_context(tc.tile_pool(name="scp", bufs=3))
    z8p = ctx.enter_context(tc.tile_pool(name="z8p", bufs=2))
    op = ctx.enter_context(tc.tile_pool(name="op", bufs=2))
    sp = ctx.enter_context(tc.tile_pool(name="sp", bufs=2))
    qwp = ctx.enter_context(tc.tile_pool(name="qwp", bufs=max(2, len(tiles))))

    x8T_r = x8T.rearrange("(kc p) t -> p kc t", p=128)
    xbT_r = xbT.rearrange("(kc p) t -> p kc t", p=128)
    outT_r = outT.rearrange("(kc p) t -> p kc t", p=128)

    ind_off = 0
    for ti, tl in enumerate(tiles):
        t0, nt, b0, nb = tl["t0"], tl["nt"], tl["b0"], tl["nb"]

        x8 = xp8.tile([128, KC, TOKCAP], FP8, tag="x8")
        nc.sync.dma_start(x8[:, :, 0:nt], x8T_r[:, :, t0:t0 + nt])
        xb = xpb.tile([128, KC, TOKCAP], BF16, tag="xb")
        nc.sync.dma_start(xb[:, :, 0:nt], xbT_r[:, :, t0:t0 + nt])

        ind = sp.tile([nb, TOKCAP], BF16, tag="ind")
        nc.sync.dma_start(ind[:, 0:nt], bass.AP(tensor=ind_blob.tensor,
                                                offset=ind_blob.offset + ind_off,
                                                ap=[[nt, nb], [1, nt]]))
        ind_off += nb * nt
        qw_loc = qwp.tile([nb, D], BF16, tag="qwloc")
        nc.sync.dma_start(qw_loc[:], qw_scr[b0:b0 + nb, :])

        # mm1 (fp8 DR): h1 = relu((x @ W1q)/s1 + b1)
        h1 = hp.tile([128, KH, TOKCAP], FP8, tag="h1")
        for mc in range(KH):
            sl = slice(mc * 128, (mc + 1) * 128)
            ps = psum.tile([128, TOKCAP], F32, tag="mmps", bufs=4)
            for k2 in range(KC // 2):
                nc.tensor.matmul(ps[:, 0:nt], W1q[:, 2 * k2:2 * k2 + 2, sl],
                                 x8[:, 2 * k2:2 * k2 + 2, 0:nt],
                                 start=(k2 == 0), stop=(k2 == KC // 2 - 1),
                                 perf_mode=DR)
            nc.scalar.activation(h1[:, mc, 0:nt], ps[:, 0:nt], AF.Relu,
                                 bias=b_mi1c[:, mc:mc + 1], scale=float(1.0 / s1))

        # mm2 (fp8 DR): r = (h1 @ W2q)/s2 + b2 + x
        r_bf = rp.tile([128, KC, TOKCAP], BF16, tag="r")
        for mc in range(KC):
            sl = slice(mc * 128, (mc + 1) * 128)
            ps = psum.tile([128, TOKCAP], F32, tag="mmps", bufs=4)
            for k2 in range(KH // 2):
                nc.tensor.matmul(ps[:, 0:nt], W2q[:, 2 * k2:2 * k2 + 2, sl],
                                 h1[:, 2 * k2:2 * k2 + 2, 0:nt],
                                 start=(k2 == 0), stop=(k2 == KH // 2 - 1),
                                 perf_mode=DR)
            tmp = sp.tile([128, TOKCAP], BF16, tag="mm2tmp")
            nc.vector.tensor_scalar(tmp[:, 0:nt], ps[:, 0:nt], float(1.0 / s2),
                                    b_mi2c[:, mc:mc + 1], op0=ALU.mult, op1=ALU.add)
            nc.vector.tensor_add(r_bf[:, mc, 0:nt], tmp[:, 0:nt], xb[:, mc, 0:nt])

        # attention: logits, mask, fused reduce+broadcast on PE, sigmoid
        at = psum.tile([nb, TOKCAP], F32, tag="atps", bufs=2)
        for kc in range(KC):
            nc.tensor.matmul(at[:, 0:nt], qT_bf[:, kc, b0:b0 + nb], r_bf[:, kc, 0:nt],
                             start=(kc == 0), stop=(kc == KC - 1))
        masked = sp.tile([nb, TOKCAP], BF16, tag="msk")
        nc.vector.tensor_tensor(masked[:, 0:nt], at[:, 0:nt], ind[:, 0:nt], op=ALU.mult)
        wb_ps = psum.tile([128, TOKCAP], F32, tag="wbps", bufs=1)
        nc.tensor.matmul(wb_ps[:, 0:nt], ones_bc[0:nb, :], masked[:, 0:nt],
                         start=True, stop=True)
        w_bc = sp.tile([128, TOKCAP], BF16, tag="wbc")
        nc.scalar.activation(w_bc[:, 0:nt], wb_ps[:, 0:nt], AF.Sigmoid, scale=RSQD)

        # wc = w * r (bf16) and its fp8 copy for mm3
        wc_bf = wcp.tile([128, KC, TOKCAP], BF16, tag="wc")
        wc8 = wc8p.tile([128, KC, TOKCAP], FP8, tag="wc8")
        for kc in range(KC):
            nc.vector.tensor_mul(wc_bf[:, kc, 0:nt], r_bf[:, kc, 0:nt], w_bc[:, 0:nt])
            nc.gpsimd.tensor_scalar_mul(wc8[:, kc, 0:nt], wc_bf[:, kc, 0:nt], 1.0)

        # mm3 (fp8 DR): scaling = tanh((wc @ W3q)/s3 + sc_b); z = wc*scaling (fp8)
        z8 = z8p.tile([128, KC, TOKCAP], FP8, tag="z8")
        for mc in range(KC):
            sl = slice(mc * 128, (mc + 1) * 128)
            ps = psum.tile([128, TOKCAP], F32, tag="mmps", bufs=4)
            for k2 in range(KC // 2):
                nc.tensor.matmul(ps[:, 0:nt], W3q[:, 2 * k2:2 * k2 + 2, sl],
                                 wc8[:, 2 * k2:2 * k2 + 2, 0:nt],
                                 start=(k2 == 0), stop=(k2 == KC // 2 - 1),
                                 perf_mode=DR)
            sc = scp.tile([128, TOKCAP], BF16, tag="sc")
            nc.scalar.activation(sc[:, 0:nt], ps[:, 0:nt], AF.Tanh,
                                 bias=b_scc[:, mc:mc + 1], scale=float(1.0 / s3))
            nc.vector.tensor_mul(z8[:, mc, 0:nt], wc_bf[:, mc, 0:nt], sc[:, 0:nt])

        # mm4 (fp8 DR + bf16 seg term): out = relu((z@W4q + s4*qw@ind)/s4 + f1_b)
        o_bf = op.tile([128, KC, TOKCAP], BF16, tag="o")
        for mc in range(KC):
            sl = slice(mc * 128, (mc + 1) * 128)
            ps = psum.tile([128, TOKCAP], F32, tag="mmps", bufs=4)
            for k2 in range(KC // 2):
                nc.tensor.matmul(ps[:, 0:nt], W4q[:, 2 * k2:2 * k2 + 2, sl],
                                 z8[:, 2 * k2:2 * k2 + 2, 0:nt],
                                 start=(k2 == 0), stop=False, perf_mode=DR)
            nc.tensor.matmul(ps[:, 0:nt], qw_loc[:, sl], ind[:, 0:nt],
                             start=False, stop=True)
            nc.scalar.activation(o_bf[:, mc, 0:nt], ps[:, 0:nt], AF.Relu,
                                 bias=b_f1c[:, mc:mc + 1], scale=float(1.0 / s4))
        nc.scalar.dma_start(outT_r[:, :, t0:t0 + nt], o_bf[:, :, 0:nt])


def _build(plan, scales):
    nc = bacc.Bacc("TRN2", target_bir_lowering=False, debug=False)
    ctx = ExitStack()
    with tile.TileContext(nc) as tc, ctx:
        _emit(ctx, tc, plan, scales)
    nc.compile()
    return nc


_NC_CACHE = {}


def _get_nc(plan_key, plan, scales):
    if plan_key not in _NC_CACHE:
        _NC_CACHE[plan_key] = _build(plan, scales)
    return _NC_CACHE[plan_key]


def _build_ind_blob(tiles):
    sz = sum(t["nb"] * t["nt"] for t in tiles)
    blob = np.zeros(sz, dtype=NPBF16)
    off = 0
    for t in tiles:
        ind = np.zeros((t["nb"], t["nt"]), dtype=NPBF16)
        for row, lo, hi in t["segs"]:
            ind[row, lo:hi] = 1
        blob[off:off + ind.size] = ind.ravel()
        off += ind.size
    return blob


def _run_cores(ncs, in_maps, trace=False):
    """Dispatch one compiled program per core, concurrently."""
    import jax
    from concourse import bass2jax
    from concourse.bass2jax import _bass_exec_p, install_neuronx_cc_hook

    install_neuronx_cc_hook()
    devices = jax.devices()[:NCORES]

    def make_jit(nc):
        in_names, out_names, out_avals, zero_outs = [], [], [], []
        for alloc in nc.m.functions[0].allocations:
            if not isinstance(alloc, mybir.MemoryLocationSet):
                continue
            name = alloc.memorylocations[0].name
            if alloc.kind == "ExternalInput":
                in_names.append(name)
            elif alloc.kind == "ExternalOutput":
                out_names.append(name)
                shape = tuple(alloc.tensor_shape)
                dtype = mybir.dt.np(alloc.dtype)
                out_avals.append(jax.core.ShapedArray(shape, dtype))
                zero_outs.append(np.zeros(shape, dtype))
        n_params = len(in_names)
        all_names = in_names + out_names

        def _body(*args):
            outs = _bass_exec_p.bind(
                *args,
                out_avals=tuple(out_avals),
                in_names=tuple(all_names),
                out_names=tuple(out_names),
                lowering_input_output_aliases=(),
                sim_require_finite=True,
                sim_require_nnan=True,
                nc=nc,
            )
            return tuple(outs)

        donate = tuple(range(n_params, n_params + len(out_names)))
        return (jax.jit(_body, donate_argnums=donate, keep_unused=True),
                in_names, out_names, zero_outs)

    with ThreadPoolExecutor(NCORES) as ex:
        jits = list(ex.map(make_jit, ncs))

    def launch(c):
        jitted, in_names, out_names, zero_outs = jits[c]
        vals = dict(in_maps[c])
        pid = ncs[c].partition_id_tensor
        if pid is not None:
            vals[pid.name] = np.array([[c]], dtype=np.uint32)
        args = [jax.device_put(np.asarray(vals[n]), devices[c]) for n in in_names]
        zz = [jax.device_put(z, devices[c]) for z in zero_outs]
        outs = jitted(*args, *zz)
        return dict(zip(out_names, outs))

    def run_all():
        with ThreadPoolExecutor(NCORES) as ex:
            outs = list(ex.map(launch, range(NCORES)))
        return [{k: np.asarray(v) for k, v in o.items()} for o in outs]

    global LAST_EXEC_NS, _LAST_TRACE
    if trace:
        import glob as globmod
        import tempfile
        from antenv.axon_hooks import get_axon_ntff_profile_hook
        hook = get_axon_ntff_profile_hook()
        neff_dir = tempfile.mkdtemp()
        if hook is None:
            results = run_all()
        else:
            run_all()  # warm: jit trace + NEFF compile before the profiled run
            with hook(neff_dir, [0]):
                results = run_all()
            try:
                import re
                import shutil
                import gauge.profiler
                from concourse._compat import FishPath
                ntffs = sorted(globmod.glob(os.path.join(neff_dir, "*_body*.ntff")))
                times = []
                insts_best = None
                for ntff in ntffs:
                    m = re.search(r"executable(\d+)", os.path.basename(ntff))
                    exe = m.group(1)
                    sub = os.path.join(neff_dir, f"exe{exe}")
                    os.makedirs(sub, exist_ok=True)
                    for fpath in globmod.glob(os.path.join(neff_dir, f"*executable{exe}*")):
                        if os.path.isfile(fpath):
                            shutil.copy(fpath, sub)
                    profile = gauge.profiler.Profile(
                        profile_path=FishPath(sub), kernel_dev_mode=True,
                        profile_on_exit=False, bass_kernel=ncs[0].m,
                        offline_processing=True, fname="*_body*",
                        metadata={"artifacts_path": sub})
                    pr = profile.to_perfetto(model_index=(0,))
                    if pr:
                        times.append(pr[0].exec_time_ns)
                        if pr[0].exec_time_ns == max(times):
                            insts_best = (pr[0].insts, pr[0].trace_path)
                if times:
                    LAST_EXEC_NS = max(times)
                    _LAST_TRACE = insts_best
                    print(f"per-core exec ns: {sorted(times)}", file=sys.stderr)
            except Exception as e:
                print(f"profile post-processing failed: {e!r}", file=sys.stderr)
    else:
        results = run_all()
    return results


def _wscale(W):
    m = float(np.abs(W).max())
    if m <= 0:
        return 1.0
    return float(2.0 ** np.floor(np.log2(200.0 / m)))


def _fp8q(W, s):
    return np.ascontiguousarray(
        np.clip(np.asarray(W, np.float32) * s, -240.0, 240.0).astype(NPFP8))


def kernel(rgns, Unet_segs, region_lens, mi_W1, mi_b1, mi_W2, mi_b2,
           ms_W1, ms_b1, ms_W2, ms_b2, seg_W, seg_b, ln_g, ln_b,
           sc_W, sc_b, f1_W, f1_b):
    _wire_ntff_hook()

    f = lambda a: np.ascontiguousarray(np.asarray(a, dtype=np.float32))
    bf = lambda a: np.ascontiguousarray(np.asarray(a, dtype=np.float32).astype(NPBF16))
    rgns = f(rgns)
    unet = np.asarray(Unet_segs, np.float32).reshape(B, SEG_C, 49).astype(NPBF16)
    lens = np.clip(np.asarray(region_lens).astype(np.int64), 0, R)

    s1, s2, s3, s4 = (_wscale(mi_W1), _wscale(mi_W2), _wscale(sc_W), _wscale(f1_W))
    weights = {
        "mi_W1q": _fp8q(mi_W1, s1), "mi_b1": f(mi_b1).reshape(1, H),
        "mi_W2q": _fp8q(mi_W2, s2), "mi_b2": f(mi_b2).reshape(1, D),
        "ms_W1": bf(ms_W1), "ms_b1": f(ms_b1).reshape(1, H),
        "ms_W2": bf(ms_W2), "ms_b2": f(ms_b2).reshape(1, D),
        "seg_W": bf(seg_W), "seg_b": f(seg_b).reshape(1, D),
        "ln_g": f(ln_g).reshape(1, D), "ln_b": f(ln_b).reshape(1, D),
        "sc_Wq": _fp8q(sc_W, s3), "sc_b": f(sc_b).reshape(1, D),
        "f1_Wq": _fp8q(f1_W, s4), "f1_W": bf(f1_W), "f1_b": f(f1_b).reshape(1, D),
    }

    # balanced batch assignment: 128 batches per core, equalize token counts
    order = np.argsort(-lens, kind="stable")
    loads = np.zeros(NCORES, dtype=np.int64)
    counts = np.zeros(NCORES, dtype=np.int64)
    assign = [[] for _ in range(NCORES)]
    for b in order:
        open_cores = [c for c in range(NCORES) if counts[c] < BC]
        c = min(open_cores, key=lambda c: loads[c])
        assign[c].append(int(b))
        loads[c] += int(lens[b])
        counts[c] += 1
    batches = [np.sort(np.array(a, dtype=np.int64)) for a in assign]

    rflat = rgns.reshape(B * R, D)
    in_maps, plans, vrows = [], [], []
    for c in range(NCORES):
        bl = batches[c]
        lens_c = lens[bl]
        plan = _make_plan(lens_c)
        plans.append(plan)
        rows = np.concatenate([bl[i] * R + np.arange(lens_c[i]) for i in range(BC)])
        vrows.append(rows)
        xc = rflat[rows]
        xbTc = np.ascontiguousarray(xc.astype(NPBF16).T)
        x8Tc = np.ascontiguousarray(
            np.clip(xc, -240.0, 240.0).astype(NPFP8).T)
        in_maps.append(dict(
            x8T=x8Tc,
            xbT=xbTc,
            unet=np.ascontiguousarray(unet[bl]),
            ind=_build_ind_blob(plan[0]),
            **weights,
        ))

    def plan_key(c):
        return (tuple((t["t0"], t["nt"], t["b0"], t["nb"], tuple(t["segs"]))
                      for t in plans[c][0]), (s1, s2, s3, s4))

    keys = [plan_key(c) for c in range(NCORES)]
    uniq = {}
    for c in range(NCORES):
        if keys[c] not in uniq:
            uniq[keys[c]] = None
    with ThreadPoolExecutor(min(8, len(uniq))) as ex:
        built = dict(zip(uniq.keys(),
                         ex.map(lambda k: _get_nc(k, plans[keys.index(k)], (s1, s2, s3, s4)),
                                list(uniq.keys()))))
    ncs = [built[keys[c]] for c in range(NCORES)]

    trace = bool(int(os.environ.get("BASSK_TRACE", "0")))
    results = _run_cores(ncs, in_maps, trace=trace)

    out = np.empty((B * R, D), np.float32)
    out[:] = results[0]["fillv"].reshape(1, D)
    for c in range(NCORES):
        out[vrows[c]] = results[c]["outT"].T.astype(np.float32)
    return out.reshape(B, R, D)


# revision 4
# speedup vs baseline: 1.8385x; 1.8385x over previous
"""Trainium2 Bass kernel for nn_CrossmodalFusion (B=1024, R=36, D=1024).

Data-parallel over batch across 8 NeuronCores with token-level sparsity:
the sigmoid attention mask zeroes every region token with j >=
region_lens[b]; for those tokens the output is exactly relu(f1_b). The
host compacts each core's valid tokens (in (batch, region) order), the
device processes only those (~51%), and the host scatters results back.

The four big per-token GEMMs (mi_W1, mi_W2, sc_W, f1_W) run in fp8-e4m3
DoubleRow mode (2 fp8 contraction planes per PE pass -> half the matmul
instructions of bf16). Weights are host-prescaled by a power of two into
fp8 range; the descale folds into the post-matmul activation `scale`.
All intermediate tensors stay bf16; fp8 copies exist only as matmul
inputs, so quantization error does not chain (CPU-model rel err 1.16e-2
vs the 2e-2 gate). The seg/qw path stays bf16 end-to-end.

Attention weights: masked logits are reduced AND broadcast to 128
partitions in a single ones-matmul on the PE (no DRAM bounce). Initial
loads are spread across the sync/scalar/vector/gpsimd DMA rings so tile
0's mm1 starts as soon as x+W1 land. The output is stored bf16.

Because each core's token plan differs, 8 per-core programs are compiled
(concurrently) and dispatched asynchronously, one per NeuronCore.
"""
import os
import sys
import types
from concurrent.futures import ThreadPoolExecutor
from contextlib import ExitStack

sys.path.insert(0, "/opt/trn_rl_repo")

import numpy as np
import ml_dtypes

import concourse.bass as bass
import concourse.tile as tile
from concourse import bacc, mybir
from concourse.masks import make_identity

F32 = mybir.dt.float32
BF16 = mybir.dt.bfloat16
FP8 = mybir.dt.float8e4

NPBF16 = ml_dtypes.bfloat16
NPFP8 = ml_dtypes.float8_e4m3  # TRN e4m3: max normal 240

B, R, D = 1024, 36, 1024
H = D // 2
SEG_C = 133
NCORES = 8
BC = B // NCORES            # batches per core
KC = D // 128               # 8 feature chunks
KH = H // 128               # 4 hidden chunks

TOKCAP = 512                # tokens per tile (PSUM fp32 bank width)
RSQD = float(1.0 / np.sqrt(D))

LAST_EXEC_NS = None
_LAST_TRACE = None


def _wire_ntff_hook():
    if "antenv.axon_hooks" in sys.modules:
        return
    try:
        import trn_agent_boot.trn_boot as tb
        hook = tb._ntff_profile_via_ctypes("/opt/axon/libaxon_pjrt.so")
    except Exception:
        hook = None
    mod = types.ModuleType("antenv.axon_hooks")
    _h = [hook]
    mod.set_axon_ntff_profile_hook = lambda h: _h.__setitem__(0, h)
    mod.get_axon_ntff_profile_hook = lambda: _h[0]
    sys.modules["antenv.axon_hooks"] = mod


def _make_plan(lens_c):
    """Tile plan for one core from its per-batch valid-token counts."""
    stream = []
    for lb, ln in enumerate(lens_c):
        stream.extend((lb, j) for j in range(int(ln)))
    ntokc = len(stream)
    tiles = []
    t0 = 0
    while t0 < ntokc:
        nt = 0
        b_first = stream[t0][0]
        while t0 + nt < ntokc and nt < TOKCAP:
            lb = stream[t0 + nt][0]
            if lb - b_first + 1 > 128:
                break
            nt += 1
        b_last = stream[t0 + nt - 1][0]
        segs = []
        pos = 0
        while pos < nt:
            lb = stream[t0 + pos][0]
            end = pos
            while end < nt and stream[t0 + end][0] == lb:
                end += 1
            segs.append((lb - b_first, pos, end))
            pos = end
        tiles.append(dict(t0=t0, nt=nt, b0=b_first, nb=b_last - b_first + 1, segs=segs))
        t0 += nt
    return tiles, ntokc


def _emit(ctx, tc, plan, scales):
    nc = tc.nc
    AF = mybir.ActivationFunctionType
    ALU = mybir.AluOpType
    DR = mybir.MatmulPerfMode.DoubleRow
    tiles, ntokc = plan
    s1, s2, s3, s4 = scales

    # ---- DRAM I/O -------------------------------------------------------
    x8T = nc.dram_tensor("x8T", [D, ntokc], FP8, kind="ExternalInput").ap()
    xbT = nc.dram_tensor("xbT", [D, ntokc], BF16, kind="ExternalInput").ap()
    unet = nc.dram_tensor("unet", [BC, SEG_C, 49], BF16, kind="ExternalInput").ap()
    ind_sz = sum(t["nb"] * t["nt"] for t in tiles)
    ind_blob = nc.dram_tensor("ind", [ind_sz], BF16, kind="ExternalInput").ap()
    wi = {}
    for name, shape, dt in [
        ("mi_W1q", [D, H], FP8), ("mi_b1", [1, H], F32),
        ("mi_W2q", [H, D], FP8), ("mi_b2", [1, D], F32),
        ("ms_W1", [D, H], BF16), ("ms_b1", [1, H], F32),
        ("ms_W2", [H, D], BF16), ("ms_b2", [1, D], F32),
        ("seg_W", [SEG_C, D], BF16), ("seg_b", [1, D], F32),
        ("ln_g", [1, D], F32), ("ln_b", [1, D], F32),
        ("sc_Wq", [D, D], FP8), ("sc_b", [1, D], F32),
        ("f1_Wq", [D, D], FP8), ("f1_W", [D, D], BF16), ("f1_b", [1, D], F32),
    ]:
        wi[name] = nc.dram_tensor(name, shape, dt, kind="ExternalInput").ap()
    outT = nc.dram_tensor("outT", [D, ntokc], BF16, kind="ExternalOutput").ap()
    fillv = nc.dram_tensor("fillv", [1, D], F32, kind="ExternalOutput").ap()

    qw_scr = nc.dram_tensor("qw_scr", [BC, D], BF16).ap()

    # ---- persistent constants ------------------------------------------
    const = ctx.enter_context(tc.tile_pool(name="const", bufs=1))

    def load_w(eng, name, kchunks, m, dt):
        t = const.tile([128, kchunks, m], dt, tag=f"cw_{name}")
        eng.dma_start(t[:], wi[name].rearrange("(kc p) m -> p kc m", p=128))
        return t

    # sync ring: what mm1/mm2 need first (then x tiles from the main loop)
    W1q = load_w(nc.sync, "mi_W1q", KC, H, FP8)
    W2q = load_w(nc.sync, "mi_W2q", KH, D, FP8)
    # scalar ring: q-stage input first, then later-needed big weights
    W3q = load_w(nc.scalar, "sc_Wq", KC, D, FP8)
    W4q = load_w(nc.scalar, "f1_Wq", KC, D, FP8)

    def load_col(name, mchunks):
        ap_ = wi[name]
        t = const.tile([128, mchunks], F32, tag=f"cc_{name}")
        src = bass.AP(tensor=ap_.tensor, offset=ap_.offset, ap=[[1, 128], [128, mchunks]])
        nc.gpsimd.dma_start(t[:], src)
        return t

    b_mi1c = load_col("mi_b1", KH)
    b_mi2c = load_col("mi_b2", KC)
    b_scc = load_col("sc_b", KC)
    b_f1c = load_col("f1_b", KC)

    ones_row = const.tile([1, 512], BF16)
    nc.vector.memset(ones_row[:], 1.0)
    ones_bc = const.tile([128, 128], BF16)   # lhsT for reduce+broadcast
    nc.vector.memset(ones_bc[:], 1.0)
    ident_bf = const.tile([128, 128], BF16)
    make_identity(nc, ident_bf)
    eps_t = const.tile([128, 1], F32)
    nc.vector.memset(eps_t[:], 1e-5)

    qT_bf = const.tile([128, KC, BC], BF16)    # feature-major q (lhsT for attn)

    psum = ctx.enter_context(tc.tile_pool(name="psum", bufs=1, space="PSUM"))

    # ============================ q-stage ================================
    with tc.tile_pool(name="qpool", bufs=1) as qp:
        unet_sb = qp.tile([BC, SEG_C, 49], BF16)
        nc.scalar.dma_start(unet_sb[:], unet[:, :, :])
        W_seg_a = qp.tile([128, D], BF16)
        nc.scalar.dma_start(W_seg_a[:], wi["seg_W"][0:128, :])
        W_seg_b = qp.tile([5, D], BF16)
        nc.scalar.dma_start(W_seg_b[:], wi["seg_W"][128:SEG_C, :])
        W_ms1 = qp.tile([128, KC, H], BF16)
        nc.gpsimd.dma_start(W_ms1[:], wi["ms_W1"].rearrange("(kc p) m -> p kc m", p=128))
        W_ms2 = qp.tile([128, KH, D], BF16)
        nc.gpsimd.dma_start(W_ms2[:], wi["ms_W2"].rearrange("(kc p) m -> p kc m", p=128))
        W_f1b = qp.tile([128, KC, D], BF16)
        nc.scalar.dma_start(W_f1b[:], wi["f1_W"].rearrange("(kc p) m -> p kc m", p=128))
        b_segr = qp.tile([1, D], BF16)
        nc.gpsimd.dma_start(b_segr[:], wi["seg_b"])
        b_ms1r = qp.tile([1, H], BF16)
        nc.gpsimd.dma_start(b_ms1r[:], wi["ms_b1"])
        b_ms2r = qp.tile([1, D], BF16)
        nc.gpsimd.dma_start(b_ms2r[:], wi["ms_b2"])
        g_bc = qp.tile([128, D], F32)
        nc.gpsimd.dma_start(g_bc[:], bass.AP(tensor=wi["ln_g"].tensor, offset=wi["ln_g"].offset, ap=[[0, 128], [1, D]]))
        bb_bc = qp.tile([128, D], F32)
        nc.gpsimd.dma_start(bb_bc[:], bass.AP(tensor=wi["ln_b"].tensor, offset=wi["ln_b"].offset, ap=[[0, 128], [1, D]]))

        # avgpool(7x7): reduce, scale, PE-transpose
        pooled = qp.tile([BC, SEG_C], F32)
        nc.vector.reduce_sum(pooled[:], unet_sb[:], axis=mybir.AxisListType.X)
        pooled_bf = qp.tile([BC, SEG_C], BF16)
        nc.scalar.mul(pooled_bf[:], pooled[:], 1.0 / 49.0)
        pa_ps = psum.tile([128, BC], BF16, tag="tps", bufs=1)
        nc.tensor.transpose(pa_ps[:], pooled_bf[:, 0:128], ident_bf[0:BC, 0:BC])
        pa_bf = qp.tile([128, BC], BF16)
        nc.scalar.copy(pa_bf[:], pa_ps[:])
        pb_ps = psum.tile([5, BC], BF16, tag="tps", bufs=1)
        nc.tensor.transpose(pb_ps[:], pooled_bf[:, 128:SEG_C], ident_bf[0:BC, 0:BC])
        pb_bf = qp.tile([5, BC], BF16)
        nc.scalar.copy(pb_bf[:], pb_ps[:])

        # q1 = relu(pooled @ seg_W + seg_b)   (token-major: BC x D)
        q1 = qp.tile([BC, D], F32)
        for ng in range(2):
            sl = slice(ng * 512, (ng + 1) * 512)
            ps = psum.tile([BC, 512], F32, tag="mmps", bufs=4)
            nc.tensor.matmul(ps[:], pa_bf[:], W_seg_a[:, sl], start=True, stop=False)
            nc.tensor.matmul(ps[:], pb_bf[:], W_seg_b[:, sl], start=False, stop=False)
            nc.tensor.matmul(ps[:], ones_row[0:1, 0:BC], b_segr[0:1, sl], start=False, stop=True)
            nc.vector.tensor_scalar_max(q1[:, sl], ps[:], 0.0)

        # layernorm over D
        stats = qp.tile([BC, 2, 6], F32)
        for s in range(2):
            nc.vector.bn_stats(stats[:, s, :], q1[:, s * 512:(s + 1) * 512])
        mv = qp.tile([BC, 2], F32)
        nc.vector.bn_aggr(mv[:], stats[:])
        rstd = qp.tile([BC, 1], F32)
        nc.scalar.activation(rstd[:], mv[:, 1:2], AF.Sqrt, bias=eps_t[0:BC, :])
        nc.vector.reciprocal(rstd[:], rstd[:])
        qn = qp.tile([BC, D], F32)
        nc.vector.tensor_scalar(qn[:], q1[:], mv[:, 0:1], rstd[:],
                                op0=ALU.subtract, op1=ALU.mult)
        nc.vector.tensor_mul(qn[:], qn[:], g_bc[0:BC, :])
        qn_bf = qp.tile([BC, D], BF16)
        nc.vector.tensor_add(qn_bf[:], qn[:], bb_bc[0:BC, :])

        # qnT (feature-major) via PE transposes
        qnT_bf = qp.tile([128, KC, BC], BF16)
        for kc in range(KC):
            pt = psum.tile([128, BC], BF16, tag="tps", bufs=1)
            nc.tensor.transpose(pt[:], qn_bf[:, kc * 128:(kc + 1) * 128], ident_bf[0:BC, 0:BC])
            nc.scalar.copy(qnT_bf[:, kc, :], pt[:])

        # q MLP (feature-major): qm = relu(ms_W1.T @ qnT + b1)
        qmT_bf = qp.tile([128, KH, BC], BF16)
        for mc in range(KH):
            sl = slice(mc * 128, (mc + 1) * 128)
            ps = psum.tile([128, BC], F32, tag="mmps", bufs=4)
            for kc in range(KC):
                nc.tensor.matmul(ps[:], W_ms1[:, kc, sl], qnT_bf[:, kc, :],
                                 start=(kc == 0), stop=False)
            nc.tensor.matmul(ps[:], b_ms1r[0:1, sl], ones_row[0:1, 0:BC],
                             start=False, stop=True)
            nc.scalar.activation(qmT_bf[:, mc, :], ps[:], AF.Relu)
        # q2T = ms_W2.T @ qmT + b2 + qnT   -> qT_bf
        for mc in range(KC):
            sl = slice(mc * 128, (mc + 1) * 128)
            ps = psum.tile([128, BC], F32, tag="mmps", bufs=4)
            for kc in range(KH):
                nc.tensor.matmul(ps[:], W_ms2[:, kc, sl], qmT_bf[:, kc, :],
                                 start=(kc == 0), stop=False)
            nc.tensor.matmul(ps[:], b_ms2r[0:1, sl], ones_row[0:1, 0:BC],
                             start=False, stop=True)
            nc.vector.tensor_add(qT_bf[:, mc, :], ps[:], qnT_bf[:, mc, :])

        # qw = s4 * (q2 @ f1_W)  (token-major bf16, prescaled to match fp8 psum)
        qw_bf = qp.tile([BC, D], BF16)
        for ng in range(2):
            sl = slice(ng * 512, (ng + 1) * 512)
            ps = psum.tile([BC, 512], F32, tag="mmps", bufs=4)
            for kc in range(KC):
                nc.tensor.matmul(ps[:], qT_bf[:, kc, :], W_f1b[:, kc, sl],
                                 start=(kc == 0), stop=(kc == KC - 1))
            nc.scalar.activation(qw_bf[:, sl], ps[:], AF.Identity, scale=float(s4))
        nc.sync.dma_start(qw_scr[:, :], qw_bf[:])

        # fill vector for masked tokens: relu(f1_b)
        fb_row = qp.tile([1, D], F32)
        nc.gpsimd.dma_start(fb_row[:], wi["f1_b"])
        fb_out = qp.tile([1, D], F32)
        nc.vector.tensor_scalar_max(fb_out[:], fb_row[:], 0.0)
        nc.sync.dma_start(fillv[:, :], fb_out[:])

    # ============================ main loop ==============================
    xp8 = ctx.enter_context(tc.tile_pool(name="xp8", bufs=2))
    xpb = ctx.enter_context(tc.tile_pool(name="xpb", bufs=2))
    hp = ctx.enter_context(tc.tile_pool(name="hp", bufs=2))
    rp = ctx.enter_context(tc.tile_pool(name="rp", bufs=2))
    wcp = ctx.enter_context(tc.tile_pool(name="wcp", bufs=2))
    wc8p = ctx.enter_context(tc.tile_pool(name="wc8p", bufs=2))
    scp = ctx.enter_context(tc.tile_pool(name="scp", bufs=3))
    z8p = ctx.enter_context(tc.tile_pool(name="z8p", bufs=2))
    op = ctx.enter_context(tc.tile_pool(name="op", bufs=2))
    sp = ctx.enter_context(tc.tile_pool(name="sp", bufs=2))
    qwp = ctx.enter_context(tc.tile_pool(name="qwp", bufs=max(2, len(tiles))))

    x8T_r = x8T.rearrange("(kc p) t -> p kc t", p=128)
    xbT_r = xbT.rearrange("(kc p) t -> p kc t", p=128)
    outT_r = outT.rearrange("(kc p) t -> p kc t", p=128)

    ind_off = 0
    for ti, tl in enumerate(tiles):
        t0, nt, b0, nb = tl["t0"], tl["nt"], tl["b0"], tl["nb"]

        x8 = xp8.tile([128, KC, TOKCAP], FP8, tag="x8")
        nc.sync.dma_start(x8[:, :, 0:nt], x8T_r[:, :, t0:t0 + nt])
        xb = xpb.tile([128, KC, TOKCAP], BF16, tag="xb")
        nc.sync.dma_start(xb[:, :, 0:nt], xbT_r[:, :, t0:t0 + nt])

        ind = sp.tile([nb, TOKCAP], BF16, tag="ind")
        nc.sync.dma_start(ind[:, 0:nt], bass.AP(tensor=ind_blob.tensor,
                                                offset=ind_blob.offset + ind_off,
                                                ap=[[nt, nb], [1, nt]]))
        ind_off += nb * nt
        qw_loc = qwp.tile([nb, D], BF16, tag="qwloc")
        nc.sync.dma_start(qw_loc[:], qw_scr[b0:b0 + nb, :])

        # mm1 (fp8 DR): h1 = relu((x @ W1q)/s1 + b1)
        h1 = hp.tile([128, KH, TOKCAP], FP8, tag="h1")
        for mc in range(KH):
            sl = slice(mc * 128, (mc + 1) * 128)
            ps = psum.tile([128, TOKCAP], F32, tag="mmps", bufs=4)
            for k2 in range(KC // 2):
                nc.tensor.matmul(ps[:, 0:nt], W1q[:, 2 * k2:2 * k2 + 2, sl],
                                 x8[:, 2 * k2:2 * k2 + 2, 0:nt],
                                 start=(k2 == 0), stop=(k2 == KC // 2 - 1),
                                 perf_mode=DR)
            nc.scalar.activation(h1[:, mc, 0:nt], ps[:, 0:nt], AF.Relu,
                                 bias=b_mi1c[:, mc:mc + 1], scale=float(1.0 / s1))

        # mm2 (fp8 DR): r = (h1 @ W2q)/s2 + b2 + x
        r_bf = rp.tile([128, KC, TOKCAP], BF16, tag="r")
        for mc in range(KC):
            sl = slice(mc * 128, (mc + 1) * 128)
            ps = psum.tile([128, TOKCAP], F32, tag="mmps", bufs=4)
            for k2 in range(KH // 2):
                nc.tensor.matmul(ps[:, 0:nt], W2q[:, 2 * k2:2 * k2 + 2, sl],
                                 h1[:, 2 * k2:2 * k2 + 2, 0:nt],
                                 start=(k2 == 0), stop=(k2 == KH // 2 - 1),
                                 perf_mode=DR)
            tmp = sp.tile([128, TOKCAP], BF16, tag="mm2tmp")
            nc.vector.tensor_scalar(tmp[:, 0:nt], ps[:, 0:nt], float(1.0 / s2),
                                    b_mi2c[:, mc:mc + 1], op0=ALU.mult, op1=ALU.add)
            nc.vector.tensor_add(r_bf[:, mc, 0:nt], tmp[:, 0:nt], xb[:, mc, 0:nt])

        # attention: logits, mask, fused reduce+broadcast on PE, sigmoid
        at = psum.tile([nb, TOKCAP], F32, tag="atps", bufs=2)
        for kc in range(KC):
            nc.tensor.matmul(at[:, 0:nt], qT_bf[:, kc, b0:b0 + nb], r_bf[:, kc, 0:nt],
                             start=(kc == 0), stop=(kc == KC - 1))
        masked = sp.tile([nb, TOKCAP], BF16, tag="msk")
        nc.vector.tensor_tensor(masked[:, 0:nt], at[:, 0:nt], ind[:, 0:nt], op=ALU.mult)
        wb_ps = psum.tile([128, TOKCAP], F32, tag="wbps", bufs=1)
        nc.tensor.matmul(wb_ps[:, 0:nt], ones_bc[0:nb, :], masked[:, 0:nt],
                         start=True, stop=True)
        w_bc = sp.tile([128, TOKCAP], BF16, tag="wbc")
        nc.scalar.activation(w_bc[:, 0:nt], wb_ps[:, 0:nt], AF.Sigmoid, scale=RSQD)

        # wc = w * r (bf16 on DVE) and its fp8 copy for mm3 (fp8 writes are
        # fast only on ScalarE; DVE/GpSimd fp8 output is ~5-15x slower)
        wc_bf = wcp.tile([128, KC, TOKCAP], BF16, tag="wc")
        wc8 = wc8p.tile([128, KC, TOKCAP], FP8, tag="wc8")
        for kc in range(KC):
            nc.vector.tensor_mul(wc_bf[:, kc, 0:nt], r_bf[:, kc, 0:nt], w_bc[:, 0:nt])
            nc.scalar.copy(wc8[:, kc, 0:nt], wc_bf[:, kc, 0:nt])

        # mm3 (fp8 DR): scaling = tanh((wc @ W3q)/s3 + sc_b); z = wc*scaling
        z8 = z8p.tile([128, KC, TOKCAP], FP8, tag="z8")
        for mc in range(KC):
            sl = slice(mc * 128, (mc + 1) * 128)
            ps = psum.tile([128, TOKCAP], F32, tag="mmps", bufs=4)
            for k2 in range(KC // 2):
                nc.tensor.matmul(ps[:, 0:nt], W3q[:, 2 * k2:2 * k2 + 2, sl],
                                 wc8[:, 2 * k2:2 * k2 + 2, 0:nt],
                                 start=(k2 == 0), stop=(k2 == KC // 2 - 1),
                                 perf_mode=DR)
            sc = scp.tile([128, TOKCAP], BF16, tag="sc")
            nc.scalar.activation(sc[:, 0:nt], ps[:, 0:nt], AF.Tanh,
                                 bias=b_scc[:, mc:mc + 1], scale=float(1.0 / s3))
            z_bf = scp.tile([128, TOKCAP], BF16, tag="zbf")
            nc.vector.tensor_mul(z_bf[:, 0:nt], wc_bf[:, mc, 0:nt], sc[:, 0:nt])
            nc.scalar.copy(z8[:, mc, 0:nt], z_bf[:, 0:nt])

        # mm4 (fp8 DR + bf16 seg term): out = relu((z@W4q + s4*qw@ind)/s4 + f1_b)
        o_bf = op.tile([128, KC, TOKCAP], BF16, tag="o")
        for mc in range(KC):
            sl = slice(mc * 128, (mc + 1) * 128)
            ps = psum.tile([128, TOKCAP], F32, tag="mmps", bufs=4)
            for k2 in range(KC // 2):
                nc.tensor.matmul(ps[:, 0:nt], W4q[:, 2 * k2:2 * k2 + 2, sl],
                                 z8[:, 2 * k2:2 * k2 + 2, 0:nt],
                                 start=(k2 == 0), stop=False, perf_mode=DR)
            nc.tensor.matmul(ps[:, 0:nt], qw_loc[:, sl], ind[:, 0:nt],
                             start=False, stop=True)
            nc.scalar.activation(o_bf[:, mc, 0:nt], ps[:, 0:nt], AF.Relu,
                                 bias=b_f1c[:, mc:mc + 1], scale=float(1.0 / s4))
        nc.scalar.dma_start(outT_r[:, :, t0:t0 + nt], o_bf[:, :, 0:nt])


def _build(plan, scales):
    nc = bacc.Bacc("TRN2", target_bir_lowering=False, debug=False)
    ctx = ExitStack()
    with tile.TileContext(nc) as tc, ctx:
        _emit(ctx, tc, plan, scales)
    nc.compile()
    return nc


_NC_CACHE = {}


def _get_nc(plan_key, plan, scales):
    if plan_key not in _NC_CACHE:
        _NC_CACHE[plan_key] = _build(plan, scales)
    return _NC_CACHE[plan_key]


def _build_ind_blob(tiles):
    sz = sum(t["nb"] * t["nt"] for t in tiles)
    blob = np.zeros(sz, dtype=NPBF16)
    off = 0
    for t in tiles:
        ind = np.zeros((t["nb"], t["nt"]), dtype=NPBF16)
        for row, lo, hi in t["segs"]:
            ind[row, lo:hi] = 1
        blob[off:off + ind.size] = ind.ravel()
        off += ind.size
    return blob


def _run_cores(ncs, in_maps, trace=False):
    """Dispatch one compiled program per core, concurrently."""
    import jax
    from concourse import bass2jax
    from concourse.bass2jax import _bass_exec_p, install_neuronx_cc_hook

    install_neuronx_cc_hook()
    devices = jax.devices()[:NCORES]

    def make_jit(nc):
        in_names, out_names, out_avals, zero_outs = [], [], [], []
        for alloc in nc.m.functions[0].allocations:
            if not isinstance(alloc, mybir.MemoryLocationSet):
                continue
            name = alloc.memorylocations[0].name
            if alloc.kind == "ExternalInput":
                in_names.append(name)
            elif alloc.kind == "ExternalOutput":
                out_names.append(name)
                shape = tuple(alloc.tensor_shape)
                dtype = mybir.dt.np(alloc.dtype)
                out_avals.append(jax.core.ShapedArray(shape, dtype))
                zero_outs.append(np.zeros(shape, dtype))
        n_params = len(in_names)
        all_names = in_names + out_names

        def _body(*args):
            outs = _bass_exec_p.bind(
                *args,
                out_avals=tuple(out_avals),
                in_names=tuple(all_names),
                out_names=tuple(out_names),
                lowering_input_output_aliases=(),
                sim_require_finite=True,
                sim_require_nnan=True,
                nc=nc,
            )
            return tuple(outs)

        donate = tuple(range(n_params, n_params + len(out_names)))
        return (jax.jit(_body, donate_argnums=donate, keep_unused=True),
                in_names, out_names, zero_outs)

    with ThreadPoolExecutor(NCORES) as ex:
        jits = list(ex.map(make_jit, ncs))

    def launch(c):
        jitted, in_names, out_names, zero_outs = jits[c]
        vals = dict(in_maps[c])
        pid = ncs[c].partition_id_tensor
        if pid is not None:
            vals[pid.name] = np.array([[c]], dtype=np.uint32)
        args = [jax.device_put(np.asarray(vals[n]), devices[c]) for n in in_names]
        zz = [jax.device_put(z, devices[c]) for z in zero_outs]
        outs = jitted(*args, *zz)
        return dict(zip(out_names, outs))

    def run_all():
        with ThreadPoolExecutor(NCORES) as ex:
            outs = list(ex.map(launch, range(NCORES)))
        return [{k: np.asarray(v) for k, v in o.items()} for o in outs]

    global LAST_EXEC_NS, _LAST_TRACE
    if trace:
        import glob as globmod
        import tempfile
        from antenv.axon_hooks import get_axon_ntff_profile_hook
        hook = get_axon_ntff_profile_hook()
        neff_dir = tempfile.mkdtemp()
        if hook is None:
            results = run_all()
        else:
            run_all()  # warm: jit trace + NEFF compile before the profiled run
            with hook(neff_dir, [0]):
                results = run_all()
            try:
                import re
                import shutil
                import gauge.profiler
                from concourse._compat import FishPath
                ntffs = sorted(globmod.glob(os.path.join(neff_dir, "*_body*.ntff")))
                times = []
                insts_best = None
                for ntff in ntffs:
                    m = re.search(r"executable(\d+)", os.path.basename(ntff))
                    exe = m.group(1)
                    sub = os.path.join(neff_dir, f"exe{exe}")
                    os.makedirs(sub, exist_ok=True)
                    for fpath in globmod.glob(os.path.join(neff_dir, f"*executable{exe}*")):
                        if os.path.isfile(fpath):
                            shutil.copy(fpath, sub)
                    profile = gauge.profiler.Profile(
                        profile_path=FishPath(sub), kernel_dev_mode=True,
                        profile_on_exit=False, bass_kernel=ncs[0].m,
                        offline_processing=True, fname="*_body*",
                        metadata={"artifacts_path": sub})
                    pr = profile.to_perfetto(model_index=(0,))
                    if pr:
                        times.append(pr[0].exec_time_ns)
                        if pr[0].exec_time_ns == max(times):
                            insts_best = (pr[0].insts, pr[0].trace_path)
                if times:
                    LAST_EXEC_NS = max(times)
                    _LAST_TRACE = insts_best
                    print(f"per-core exec ns: {sorted(times)}", file=sys.stderr)
            except Exception as e:
                print(f"profile post-processing failed: {e!r}", file=sys.stderr)
    else:
        results = run_all()
    return results


def _wscale(W):
    m = float(np.abs(W).max())
    if m <= 0:
        return 1.0
    return float(2.0 ** np.floor(np.log2(200.0 / m)))


def _fp8q(W, s):
    return np.ascontiguousarray(
        np.clip(np.asarray(W, np.float32) * s, -240.0, 240.0).astype(NPFP8))


def kernel(rgns, Unet_segs, region_lens, mi_W1, mi_b1, mi_W2, mi_b2,
           ms_W1, ms_b1, ms_W2, ms_b2, seg_W, seg_b, ln_g, ln_b,
           sc_W, sc_b, f1_W, f1_b):
    _wire_ntff_hook()

    f = lambda a: np.ascontiguousarray(np.asarray(a, dtype=np.float32))
    bf = lambda a: np.ascontiguousarray(np.asarray(a, dtype=np.float32).astype(NPBF16))
    rgns = f(rgns)
    unet = np.asarray(Unet_segs, np.float32).reshape(B, SEG_C, 49).astype(NPBF16)
    lens = np.clip(np.asarray(region_lens).astype(np.int64), 0, R)

    s1, s2, s3, s4 = (_wscale(mi_W1), _wscale(mi_W2), _wscale(sc_W), _wscale(f1_W))
    weights = {
        "mi_W1q": _fp8q(mi_W1, s1), "mi_b1": f(mi_b1).reshape(1, H),
        "mi_W2q": _fp8q(mi_W2, s2), "mi_b2": f(mi_b2).reshape(1, D),
        "ms_W1": bf(ms_W1), "ms_b1": f(ms_b1).reshape(1, H),
        "ms_W2": bf(ms_W2), "ms_b2": f(ms_b2).reshape(1, D),
        "seg_W": bf(seg_W), "seg_b": f(seg_b).reshape(1, D),
        "ln_g": f(ln_g).reshape(1, D), "ln_b": f(ln_b).reshape(1, D),
        "sc_Wq": _fp8q(sc_W, s3), "sc_b": f(sc_b).reshape(1, D),
        "f1_Wq": _fp8q(f1_W, s4), "f1_W": bf(f1_W), "f1_b": f(f1_b).reshape(1, D),
    }

    # balanced batch assignment: 128 batches per core, equalize token counts
    order = np.argsort(-lens, kind="stable")
    loads = np.zeros(NCORES, dtype=np.int64)
    counts = np.zeros(NCORES, dtype=np.int64)
    assign = [[] for _ in range(NCORES)]
    for b in order:
        open_cores = [c for c in range(NCORES) if counts[c] < BC]
        c = min(open_cores, key=lambda c: loads[c])
        assign[c].append(int(b))
        loads[c] += int(lens[b])
        counts[c] += 1
    batches = [np.sort(np.array(a, dtype=np.int64)) for a in assign]

    rflat = rgns.reshape(B * R, D)
    in_maps, plans, vrows = [], [], []
    for c in range(NCORES):
        bl = batches[c]
        lens_c = lens[bl]
        plan = _make_plan(lens_c)
        plans.append(plan)
        rows = np.concatenate([bl[i] * R + np.arange(lens_c[i]) for i in range(BC)])
        vrows.append(rows)
        xc = rflat[rows]
        xbTc = np.ascontiguousarray(xc.astype(NPBF16).T)
        x8Tc = np.ascontiguousarray(
            np.clip(xc, -240.0, 240.0).astype(NPFP8).T)
        in_maps.append(dict(
            x8T=x8Tc,
            xbT=xbTc,
            unet=np.ascontiguousarray(unet[bl]),
            ind=_build_ind_blob(plan[0]),
            **weights,
        ))

    def plan_key(c):
        return (tuple((t["t0"], t["nt"], t["b0"], t["nb"], tuple(t["segs"]))
                      for t in plans[c][0]), (s1, s2, s3, s4))

    keys = [plan_key(c) for c in range(NCORES)]
    uniq = {}
    for c in range(NCORES):
        if keys[c] not in uniq:
            uniq[keys[c]] = None
    with ThreadPoolExecutor(min(8, len(uniq))) as ex:
        built = dict(zip(uniq.keys(),
                         ex.map(lambda k: _get_nc(k, plans[keys.index(k)], (s1, s2, s3, s4)),
                                list(uniq.keys()))))
    ncs = [built[keys[c]] for c in range(NCORES)]

    trace = bool(int(os.environ.get("BASSK_TRACE", "0")))
    results = _run_cores(ncs, in_maps, trace=trace)

    out = np.empty((B * R, D), np.float32)
    out[:] = results[0]["fillv"].reshape(1, D)
    for c in range(NCORES):
        out[vrows[c]] = results[c]["outT"].T.astype(np.float32)
    return out.reshape(B, R, D)


# revision 8
# speedup vs baseline: 2.0164x; 1.0968x over previous
"""Trainium2 Bass kernel for nn_CrossmodalFusion (B=1024, R=36, D=1024).

Data-parallel over batch across 8 NeuronCores with token-level sparsity:
the sigmoid attention mask zeroes every region token with j >=
region_lens[b]; for those tokens the output is exactly relu(f1_b). The
host compacts each core's valid tokens (in (batch, region) order), the
device processes only those (~51%), and the host scatters results back.

The four big per-token GEMMs (mi_W1, mi_W2, sc_W, f1_W) run in fp8-e4m3
DoubleRow mode (2 fp8 contraction planes per PE pass -> half the matmul
instructions of bf16). Weights are host-prescaled by a power of two into
fp8 range; the descale folds into the post-matmul activation `scale`.
All intermediate tensors stay bf16; fp8 copies exist only as matmul
inputs, so quantization error does not chain (CPU-model rel err 1.16e-2
vs the 2e-2 gate). The seg/qw path stays bf16 end-to-end.

Attention weights: masked logits are reduced AND broadcast to 128
partitions in a single ones-matmul on the PE (no DRAM bounce). Initial
loads are spread across the sync/scalar/vector/gpsimd DMA rings so tile
0's mm1 starts as soon as x+W1 land. The output is stored bf16.

Because each core's token plan differs, 8 per-core programs are compiled
(concurrently) and dispatched asynchronously, one per NeuronCore.
"""
import os
import sys
import types
from concurrent.futures import ThreadPoolExecutor
from contextlib import ExitStack

sys.path.insert(0, "/opt/trn_rl_repo")

import numpy as np
import ml_dtypes

import concourse.bass as bass
import concourse.tile as tile
from concourse import bacc, mybir
from concourse.masks import make_identity

F32 = mybir.dt.float32
BF16 = mybir.dt.bfloat16
FP8 = mybir.dt.float8e4

NPBF16 = ml_dtypes.bfloat16
NPFP8 = ml_dtypes.float8_e4m3  # TRN e4m3: max normal 240

B, R, D = 1024, 36, 1024
H = D // 2
SEG_C = 133
NCORES = 8
BC = B // NCORES            # batches per core
KC = D // 128               # 8 feature chunks
KH = H // 128               # 4 hidden chunks

TOKCAP = 512                # tokens per tile (PSUM fp32 bank width)
RSQD = float(1.0 / np.sqrt(D))

LAST_EXEC_NS = None
_LAST_TRACE = None


def _wire_ntff_hook():
    if "antenv.axon_hooks" in sys.modules:
        return
    try:
        import trn_agent_boot.trn_boot as tb
        hook = tb._ntff_profile_via_ctypes("/opt/axon/libaxon_pjrt.so")
    except Exception:
        hook = None
    mod = types.ModuleType("antenv.axon_hooks")
    _h = [hook]
    mod.set_axon_ntff_profile_hook = lambda h: _h.__setitem__(0, h)
    mod.get_axon_ntff_profile_hook = lambda: _h[0]
    sys.modules["antenv.axon_hooks"] = mod


def _make_plan(lens_c):
    """Tile plan for one core from its per-batch valid-token counts."""
    stream = []
    for lb, ln in enumerate(lens_c):
        stream.extend((lb, j) for j in range(int(ln)))
    ntokc = len(stream)
    tiles = []
    t0 = 0
    while t0 < ntokc:
        nt = 0
        b_first = stream[t0][0]
        while t0 + nt < ntokc and nt < TOKCAP:
            lb = stream[t0 + nt][0]
            if lb - b_first + 1 > 128:
                break
            nt += 1
        b_last = stream[t0 + nt - 1][0]
        segs = []
        pos = 0
        while pos < nt:
            lb = stream[t0 + pos][0]
            end = pos
            while end < nt and stream[t0 + end][0] == lb:
                end += 1
            segs.append((lb - b_first, pos, end))
            pos = end
        tiles.append(dict(t0=t0, nt=nt, b0=b_first, nb=b_last - b_first + 1, segs=segs))
        t0 += nt
    return tiles, ntokc


def _emit(ctx, tc, plan, scales):
    nc = tc.nc
    AF = mybir.ActivationFunctionType
    ALU = mybir.AluOpType
    DR = mybir.MatmulPerfMode.DoubleRow
    tiles, ntokc = plan
    s1, s2, s3, s4 = scales

    # ---- DRAM I/O -------------------------------------------------------
    x8T = nc.dram_tensor("x8T", [D, ntokc], FP8, kind="ExternalInput").ap()
    xbT = nc.dram_tensor("xbT", [D, ntokc], BF16, kind="ExternalInput").ap()
    unet = nc.dram_tensor("unet", [BC, SEG_C, 49], BF16, kind="ExternalInput").ap()
    ind_sz = sum(t["nb"] * t["nt"] for t in tiles)
    ind_blob = nc.dram_tensor("ind", [ind_sz], BF16, kind="ExternalInput").ap()
    wi = {}
    for name, shape, dt in [
        ("mi_W1q", [D, H], FP8), ("mi_b1", [1, H], F32),
        ("mi_W2q", [H, D], FP8), ("mi_b2", [1, D], F32),
        ("ms_W1", [D, H], BF16), ("ms_b1", [1, H], F32),
        ("ms_W2", [H, D], BF16), ("ms_b2", [1, D], F32),
        ("seg_W", [SEG_C, D], BF16), ("seg_b", [1, D], F32),
        ("ln_g", [1, D], F32), ("ln_b", [1, D], F32),
        ("sc_Wq", [D, D], FP8), ("sc_b", [1, D], F32),
        ("f1_Wq", [D, D], FP8), ("f1_W", [D, D], BF16), ("f1_b", [1, D], F32),
    ]:
        wi[name] = nc.dram_tensor(name, shape, dt, kind="ExternalInput").ap()
    outT = nc.dram_tensor("outT", [D, ntokc], BF16, kind="ExternalOutput").ap()
    fillv = nc.dram_tensor("fillv", [1, D], F32, kind="ExternalOutput").ap()

    qw_scr = nc.dram_tensor("qw_scr", [BC, D], BF16).ap()

    # ---- persistent constants ------------------------------------------
    const = ctx.enter_context(tc.tile_pool(name="const", bufs=1))

    def load_w(eng, name, kchunks, m, dt):
        t = const.tile([128, kchunks, m], dt, tag=f"cw_{name}")
        eng.dma_start(t[:], wi[name].rearrange("(kc p) m -> p kc m", p=128))
        return t

    # sync ring: what mm1/mm2 need first (then x tiles from the main loop)
    W1q = load_w(nc.sync, "mi_W1q", KC, H, FP8)
    W2q = load_w(nc.sync, "mi_W2q", KH, D, FP8)

    def load_col(name, mchunks):
        ap_ = wi[name]
        t = const.tile([128, mchunks], F32, tag=f"cc_{name}")
        src = bass.AP(tensor=ap_.tensor, offset=ap_.offset, ap=[[1, 128], [128, mchunks]])
        nc.gpsimd.dma_start(t[:], src)
        return t

    b_mi1c = load_col("mi_b1", KH)
    b_mi2c = load_col("mi_b2", KC)
    b_scc = load_col("sc_b", KC)
    b_f1c = load_col("f1_b", KC)

    ones_row = const.tile([1, 512], BF16)
    nc.vector.memset(ones_row[:], 1.0)
    ones_bc = const.tile([128, 128], BF16)   # lhsT for reduce+broadcast
    nc.vector.memset(ones_bc[:], 1.0)
    ident_bf = const.tile([128, 128], BF16)
    make_identity(nc, ident_bf)
    eps_t = const.tile([128, 1], F32)
    nc.vector.memset(eps_t[:], 1e-5)

    qT_bf = const.tile([128, KC, BC], BF16)    # feature-major q (lhsT for attn)

    psum = ctx.enter_context(tc.tile_pool(name="psum", bufs=1, space="PSUM"))

    # ============================ q-stage ================================
    with tc.tile_pool(name="qpool", bufs=1) as qp:
        unet_sb = qp.tile([BC, SEG_C, 49], BF16)
        nc.scalar.dma_start(unet_sb[:], unet[:, :, :])
        W_seg_a = qp.tile([128, D], BF16)
        nc.scalar.dma_start(W_seg_a[:], wi["seg_W"][0:128, :])
        W_seg_b = qp.tile([5, D], BF16)
        nc.scalar.dma_start(W_seg_b[:], wi["seg_W"][128:SEG_C, :])
        W_ms1 = qp.tile([128, KC, H], BF16)
        nc.gpsimd.dma_start(W_ms1[:], wi["ms_W1"].rearrange("(kc p) m -> p kc m", p=128))
        W_ms2 = qp.tile([128, KH, D], BF16)
        nc.gpsimd.dma_start(W_ms2[:], wi["ms_W2"].rearrange("(kc p) m -> p kc m", p=128))
        W_f1b = qp.tile([128, KC, D], BF16)
        nc.scalar.dma_start(W_f1b[:], wi["f1_W"].rearrange("(kc p) m -> p kc m", p=128))
        # late-needed fp8 weights go on the scalar ring AFTER the q-stage
        # inputs (ring order ~ emission order; these must land before mm3/mm4
        # of tile 0, ~45us in)
        W3q = load_w(nc.scalar, "sc_Wq", KC, D, FP8)
        W4q = load_w(nc.scalar, "f1_Wq", KC, D, FP8)
        b_segr = qp.tile([1, D], BF16)
        nc.gpsimd.dma_start(b_segr[:], wi["seg_b"])
        b_ms1r = qp.tile([1, H], BF16)
        nc.gpsimd.dma_start(b_ms1r[:], wi["ms_b1"])
        b_ms2r = qp.tile([1, D], BF16)
        nc.gpsimd.dma_start(b_ms2r[:], wi["ms_b2"])
        g_bc = qp.tile([128, D], F32)
        nc.gpsimd.dma_start(g_bc[:], bass.AP(tensor=wi["ln_g"].tensor, offset=wi["ln_g"].offset, ap=[[0, 128], [1, D]]))
        bb_bc = qp.tile([128, D], F32)
        nc.gpsimd.dma_start(bb_bc[:], bass.AP(tensor=wi["ln_b"].tensor, offset=wi["ln_b"].offset, ap=[[0, 128], [1, D]]))

        # avgpool(7x7): reduce, scale, PE-transpose
        pooled = qp.tile([BC, SEG_C], F32)
        nc.vector.reduce_sum(pooled[:], unet_sb[:], axis=mybir.AxisListType.X)
        pooled_bf = qp.tile([BC, SEG_C], BF16)
        nc.scalar.mul(pooled_bf[:], pooled[:], 1.0 / 49.0)
        pa_ps = psum.tile([128, BC], BF16, tag="tps", bufs=1)
        nc.tensor.transpose(pa_ps[:], pooled_bf[:, 0:128], ident_bf[0:BC, 0:BC])
        pa_bf = qp.tile([128, BC], BF16)
        nc.scalar.copy(pa_bf[:], pa_ps[:])
        pb_ps = psum.tile([5, BC], BF16, tag="tps", bufs=1)
        nc.tensor.transpose(pb_ps[:], pooled_bf[:, 128:SEG_C], ident_bf[0:BC, 0:BC])
        pb_bf = qp.tile([5, BC], BF16)
        nc.scalar.copy(pb_bf[:], pb_ps[:])

        # q1 = relu(pooled @ seg_W + seg_b)   (token-major: BC x D)
        q1 = qp.tile([BC, D], F32)
        for ng in range(2):
            sl = slice(ng * 512, (ng + 1) * 512)
            ps = psum.tile([BC, 512], F32, tag="mmps", bufs=4)
            nc.tensor.matmul(ps[:], pa_bf[:], W_seg_a[:, sl], start=True, stop=False)
            nc.tensor.matmul(ps[:], pb_bf[:], W_seg_b[:, sl], start=False, stop=False)
            nc.tensor.matmul(ps[:], ones_row[0:1, 0:BC], b_segr[0:1, sl], start=False, stop=True)
            nc.vector.tensor_scalar_max(q1[:, sl], ps[:], 0.0)

        # layernorm over D
        stats = qp.tile([BC, 2, 6], F32)
        for s in range(2):
            nc.vector.bn_stats(stats[:, s, :], q1[:, s * 512:(s + 1) * 512])
        mv = qp.tile([BC, 2], F32)
        nc.vector.bn_aggr(mv[:], stats[:])
        rstd = qp.tile([BC, 1], F32)
        nc.scalar.activation(rstd[:], mv[:, 1:2], AF.Sqrt, bias=eps_t[0:BC, :])
        nc.vector.reciprocal(rstd[:], rstd[:])
        qn = qp.tile([BC, D], F32)
        nc.vector.tensor_scalar(qn[:], q1[:], mv[:, 0:1], rstd[:],
                                op0=ALU.subtract, op1=ALU.mult)
        nc.vector.tensor_mul(qn[:], qn[:], g_bc[0:BC, :])
        qn_bf = qp.tile([BC, D], BF16)
        nc.vector.tensor_add(qn_bf[:], qn[:], bb_bc[0:BC, :])

        # qnT (feature-major) via PE transposes
        qnT_bf = qp.tile([128, KC, BC], BF16)
        for kc in range(KC):
            pt = psum.tile([128, BC], BF16, tag="tps", bufs=1)
            nc.tensor.transpose(pt[:], qn_bf[:, kc * 128:(kc + 1) * 128], ident_bf[0:BC, 0:BC])
            nc.scalar.copy(qnT_bf[:, kc, :], pt[:])

        # q MLP (feature-major): qm = relu(ms_W1.T @ qnT + b1)
        qmT_bf = qp.tile([128, KH, BC], BF16)
        for mc in range(KH):
            sl = slice(mc * 128, (mc + 1) * 128)
            ps = psum.tile([128, BC], F32, tag="mmps", bufs=4)
            for kc in range(KC):
                nc.tensor.matmul(ps[:], W_ms1[:, kc, sl], qnT_bf[:, kc, :],
                                 start=(kc == 0), stop=False)
            nc.tensor.matmul(ps[:], b_ms1r[0:1, sl], ones_row[0:1, 0:BC],
                             start=False, stop=True)
            nc.scalar.activation(qmT_bf[:, mc, :], ps[:], AF.Relu)
        # q2T = ms_W2.T @ qmT + b2 + qnT   -> qT_bf
        for mc in range(KC):
            sl = slice(mc * 128, (mc + 1) * 128)
            ps = psum.tile([128, BC], F32, tag="mmps", bufs=4)
            for kc in range(KH):
                nc.tensor.matmul(ps[:], W_ms2[:, kc, sl], qmT_bf[:, kc, :],
                                 start=(kc == 0), stop=False)
            nc.tensor.matmul(ps[:], b_ms2r[0:1, sl], ones_row[0:1, 0:BC],
                             start=False, stop=True)
            nc.vector.tensor_add(qT_bf[:, mc, :], ps[:], qnT_bf[:, mc, :])

        # qw = s4 * (q2 @ f1_W)  (token-major bf16, prescaled to match fp8 psum)
        qw_bf = qp.tile([BC, D], BF16)
        for ng in range(2):
            sl = slice(ng * 512, (ng + 1) * 512)
            ps = psum.tile([BC, 512], F32, tag="mmps", bufs=4)
            for kc in range(KC):
                nc.tensor.matmul(ps[:], qT_bf[:, kc, :], W_f1b[:, kc, sl],
                                 start=(kc == 0), stop=(kc == KC - 1))
            nc.scalar.activation(qw_bf[:, sl], ps[:], AF.Identity, scale=float(s4))
        nc.sync.dma_start(qw_scr[:, :], qw_bf[:])

        # fill vector for masked tokens: relu(f1_b)
        fb_row = qp.tile([1, D], F32)
        nc.gpsimd.dma_start(fb_row[:], wi["f1_b"])
        fb_out = qp.tile([1, D], F32)
        nc.vector.tensor_scalar_max(fb_out[:], fb_row[:], 0.0)
        nc.sync.dma_start(fillv[:, :], fb_out[:])

    # ============================ main loop ==============================
    xp8 = ctx.enter_context(tc.tile_pool(name="xp8", bufs=2))
    xpb = ctx.enter_context(tc.tile_pool(name="xpb", bufs=2))
    hp = ctx.enter_context(tc.tile_pool(name="hp", bufs=2))
    rp = ctx.enter_context(tc.tile_pool(name="rp", bufs=2))
    wcp = ctx.enter_context(tc.tile_pool(name="wcp", bufs=2))
    wc8p = ctx.enter_context(tc.tile_pool(name="wc8p", bufs=2))
    scp = ctx.enter_context(tc.tile_pool(name="scp", bufs=3))
    z8p = ctx.enter_context(tc.tile_pool(name="z8p", bufs=2))
    op = ctx.enter_context(tc.tile_pool(name="op", bufs=2))
    sp = ctx.enter_context(tc.tile_pool(name="sp", bufs=2))
    qwp = ctx.enter_context(tc.tile_pool(name="qwp", bufs=max(2, len(tiles))))

    x8T_r = x8T.rearrange("(kc p) t -> p kc t", p=128)
    xbT_r = xbT.rearrange("(kc p) t -> p kc t", p=128)
    outT_r = outT.rearrange("(kc p) t -> p kc t", p=128)

    # Two-stage software pipeline. Stage A(t) = loads, mm1, mm2, attention,
    # sigmoid weights, wc (+fp8 copy). Stage B(t) = mm3, z, mm4, store.
    # Emission order A(0), A(1), B(0), A(2), B(1), ... so the PE's (in-order)
    # queue can fill tile t's scalar/vector latency (sigmoid -> wc -> wc8)
    # with tile t+1's mm1/mm2 instead of idling.
    ind_offs = []
    off = 0
    for tl in tiles:
        ind_offs.append(off)
        off += tl["nb"] * tl["nt"]
    state = {}

    def emit_A(ti):
        tl = tiles[ti]
        t0, nt, b0, nb = tl["t0"], tl["nt"], tl["b0"], tl["nb"]
        x8 = xp8.tile([128, KC, TOKCAP], FP8, tag="x8")
        nc.sync.dma_start(x8[:, :, 0:nt], x8T_r[:, :, t0:t0 + nt])
        xb = xpb.tile([128, KC, TOKCAP], BF16, tag="xb")
        nc.sync.dma_start(xb[:, :, 0:nt], xbT_r[:, :, t0:t0 + nt])
        ind = sp.tile([nb, TOKCAP], BF16, tag="ind")
        nc.sync.dma_start(ind[:, 0:nt], bass.AP(tensor=ind_blob.tensor,
                                                offset=ind_blob.offset + ind_offs[ti],
                                                ap=[[nt, nb], [1, nt]]))
        qw_loc = qwp.tile([nb, D], BF16, tag="qwloc")
        nc.sync.dma_start(qw_loc[:], qw_scr[b0:b0 + nb, :])

        # mm1 (fp8 DR): h1 = relu((x @ W1q)/s1 + b1)
        h1 = hp.tile([128, KH, TOKCAP], FP8, tag="h1")
        for mc in range(KH):
            sl = slice(mc * 128, (mc + 1) * 128)
            ps = psum.tile([128, TOKCAP], F32, tag="mmps", bufs=4)
            for k2 in range(KC // 2):
                nc.tensor.matmul(ps[:, 0:nt], W1q[:, 2 * k2:2 * k2 + 2, sl],
                                 x8[:, 2 * k2:2 * k2 + 2, 0:nt],
                                 start=(k2 == 0), stop=(k2 == KC // 2 - 1),
                                 perf_mode=DR)
            nc.scalar.activation(h1[:, mc, 0:nt], ps[:, 0:nt], AF.Relu,
                                 bias=b_mi1c[:, mc:mc + 1], scale=float(1.0 / s1))

        # mm2 (fp8 DR): r = (h1 @ W2q)/s2 + b2 + x
        r_bf = rp.tile([128, KC, TOKCAP], BF16, tag="r")
        for mc in range(KC):
            sl = slice(mc * 128, (mc + 1) * 128)
            ps = psum.tile([128, TOKCAP], F32, tag="mmps", bufs=4)
            for k2 in range(KH // 2):
                nc.tensor.matmul(ps[:, 0:nt], W2q[:, 2 * k2:2 * k2 + 2, sl],
                                 h1[:, 2 * k2:2 * k2 + 2, 0:nt],
                                 start=(k2 == 0), stop=(k2 == KH // 2 - 1),
                                 perf_mode=DR)
            tmp = sp.tile([128, TOKCAP], BF16, tag="mm2tmp")
            nc.vector.tensor_scalar(tmp[:, 0:nt], ps[:, 0:nt], float(1.0 / s2),
                                    b_mi2c[:, mc:mc + 1], op0=ALU.mult, op1=ALU.add)
            nc.vector.tensor_add(r_bf[:, mc, 0:nt], tmp[:, 0:nt], xb[:, mc, 0:nt])

        # attention: logits, mask, fused reduce+broadcast on PE, sigmoid
        at = psum.tile([nb, TOKCAP], F32, tag="atps", bufs=2)
        for kc in range(KC):
            nc.tensor.matmul(at[:, 0:nt], qT_bf[:, kc, b0:b0 + nb], r_bf[:, kc, 0:nt],
                             start=(kc == 0), stop=(kc == KC - 1))
        masked = sp.tile([nb, TOKCAP], BF16, tag="msk")
        nc.vector.tensor_tensor(masked[:, 0:nt], at[:, 0:nt], ind[:, 0:nt], op=ALU.mult)
        wb_ps = psum.tile([128, TOKCAP], F32, tag="wbps", bufs=1)
        nc.tensor.matmul(wb_ps[:, 0:nt], ones_bc[0:nb, :], masked[:, 0:nt],
                         start=True, stop=True)
        w_bc = sp.tile([128, TOKCAP], BF16, tag="wbc")
        nc.scalar.activation(w_bc[:, 0:nt], wb_ps[:, 0:nt], AF.Sigmoid, scale=RSQD)

        # wc = w * r (bf16 on DVE) and its fp8 copy for mm3 (fp8 writes are
        # fast only on ScalarE; DVE/GpSimd fp8 output is ~5-15x slower)
        wc_bf = wcp.tile([128, KC, TOKCAP], BF16, tag="wc")
        wc8 = wc8p.tile([128, KC, TOKCAP], FP8, tag="wc8")
        for kc in range(KC):
            nc.vector.tensor_mul(wc_bf[:, kc, 0:nt], r_bf[:, kc, 0:nt], w_bc[:, 0:nt])
            nc.scalar.copy(wc8[:, kc, 0:nt], wc_bf[:, kc, 0:nt])
        state[ti] = dict(nt=nt, t0=t0, wc_bf=wc_bf, wc8=wc8, ind=ind, qw_loc=qw_loc)

    def emit_B(ti):
        st = state.pop(ti)
        nt, t0 = st["nt"], st["t0"]
        wc_bf, wc8, ind, qw_loc = st["wc_bf"], st["wc8"], st["ind"], st["qw_loc"]

        # mm3 (fp8 DR): scaling = tanh((wc @ W3q)/s3 + sc_b); z = wc*scaling
        z8 = z8p.tile([128, KC, TOKCAP], FP8, tag="z8")
        for mc in range(KC):
            sl = slice(mc * 128, (mc + 1) * 128)
            ps = psum.tile([128, TOKCAP], F32, tag="mmps", bufs=4)
            for k2 in range(KC // 2):
                nc.tensor.matmul(ps[:, 0:nt], W3q[:, 2 * k2:2 * k2 + 2, sl],
                                 wc8[:, 2 * k2:2 * k2 + 2, 0:nt],
                                 start=(k2 == 0), stop=(k2 == KC // 2 - 1),
                                 perf_mode=DR)
            sc = scp.tile([128, TOKCAP], BF16, tag="sc")
            nc.scalar.activation(sc[:, 0:nt], ps[:, 0:nt], AF.Tanh,
                                 bias=b_scc[:, mc:mc + 1], scale=float(1.0 / s3))
            z_bf = scp.tile([128, TOKCAP], BF16, tag="zbf")
            nc.vector.tensor_mul(z_bf[:, 0:nt], wc_bf[:, mc, 0:nt], sc[:, 0:nt])
            nc.scalar.copy(z8[:, mc, 0:nt], z_bf[:, 0:nt])

        # mm4 (fp8 DR + bf16 seg term): out = relu((z@W4q + s4*qw@ind)/s4 + f1_b)
        o_bf = op.tile([128, KC, TOKCAP], BF16, tag="o")
        for mc in range(KC):
            sl = slice(mc * 128, (mc + 1) * 128)
            ps = psum.tile([128, TOKCAP], F32, tag="mmps", bufs=4)
            for k2 in range(KC // 2):
                nc.tensor.matmul(ps[:, 0:nt], W4q[:, 2 * k2:2 * k2 + 2, sl],
                                 z8[:, 2 * k2:2 * k2 + 2, 0:nt],
                                 start=(k2 == 0), stop=False, perf_mode=DR)
            nc.tensor.matmul(ps[:, 0:nt], qw_loc[:, sl], ind[:, 0:nt],
                             start=False, stop=True)
            nc.scalar.activation(o_bf[:, mc, 0:nt], ps[:, 0:nt], AF.Relu,
                                 bias=b_f1c[:, mc:mc + 1], scale=float(1.0 / s4))
            if mc == KC // 2 - 1:
                nc.scalar.dma_start(outT_r[:, 0:KC // 2, t0:t0 + nt],
                                    o_bf[:, 0:KC // 2, 0:nt])
        nc.scalar.dma_start(outT_r[:, KC // 2:KC, t0:t0 + nt],
                            o_bf[:, KC // 2:KC, 0:nt])

    emit_A(0)
    for ti in range(1, len(tiles)):
        emit_A(ti)
        emit_B(ti - 1)
    emit_B(len(tiles) - 1)


def _build(plan, scales):
    nc = bacc.Bacc("TRN2", target_bir_lowering=False, debug=False)
    ctx = ExitStack()
    with tile.TileContext(nc) as tc, ctx:
        _emit(ctx, tc, plan, scales)
    nc.compile()
    return nc


_NC_CACHE = {}


def _get_nc(plan_key, plan, scales):
    if plan_key not in _NC_CACHE:
        _NC_CACHE[plan_key] = _build(plan, scales)
    return _NC_CACHE[plan_key]


def _build_ind_blob(tiles):
    sz = sum(t["nb"] * t["nt"] for t in tiles)
    blob = np.zeros(sz, dtype=NPBF16)
    off = 0
    for t in tiles:
        ind = np.zeros((t["nb"], t["nt"]), dtype=NPBF16)
        for row, lo, hi in t["segs"]:
            ind[row, lo:hi] = 1
        blob[off:off + ind.size] = ind.ravel()
        off += ind.size
    return blob


def _run_cores(ncs, in_maps, trace=False):
    """Dispatch one compiled program per core, concurrently."""
    import jax
    from concourse import bass2jax
    from concourse.bass2jax import _bass_exec_p, install_neuronx_cc_hook

    install_neuronx_cc_hook()
    devices = jax.devices()[:NCORES]

    def make_jit(nc):
        in_names, out_names, out_avals, zero_outs = [], [], [], []
        for alloc in nc.m.functions[0].allocations:
            if not isinstance(alloc, mybir.MemoryLocationSet):
                continue
            name = alloc.memorylocations[0].name
            if alloc.kind == "ExternalInput":
                in_names.append(name)
            elif alloc.kind == "ExternalOutput":
                out_names.append(name)
                shape = tuple(alloc.tensor_shape)
                dtype = mybir.dt.np(alloc.dtype)
                out_avals.append(jax.core.ShapedArray(shape, dtype))
                zero_outs.append(np.zeros(shape, dtype))
        n_params = len(in_names)
        all_names = in_names + out_names

        def _body(*args):
            outs = _bass_exec_p.bind(
                *args,
                out_avals=tuple(out_avals),
                in_names=tuple(all_names),
                out_names=tuple(out_names),
                lowering_input_output_aliases=(),
                sim_require_finite=True,
                sim_require_nnan=True,
                nc=nc,
            )
            return tuple(outs)

        donate = tuple(range(n_params, n_params + len(out_names)))
        return (jax.jit(_body, donate_argnums=donate, keep_unused=True),
                in_names, out_names, zero_outs)

    with ThreadPoolExecutor(NCORES) as ex:
        jits = list(ex.map(make_jit, ncs))

    def launch(c):
        jitted, in_names, out_names, zero_outs = jits[c]
        vals = dict(in_maps[c])
        pid = ncs[c].partition_id_tensor
        if pid is not None:
            vals[pid.name] = np.array([[c]], dtype=np.uint32)
        args = [jax.device_put(np.asarray(vals[n]), devices[c]) for n in in_names]
        zz = [jax.device_put(z, devices[c]) for z in zero_outs]
        outs = jitted(*args, *zz)
        return dict(zip(out_names, outs))

    def run_all():
        with ThreadPoolExecutor(NCORES) as ex:
            outs = list(ex.map(launch, range(NCORES)))
        return [{k: np.asarray(v) for k, v in o.items()} for o in outs]

    global LAST_EXEC_NS, _LAST_TRACE
    if trace:
        import glob as globmod
        import tempfile
        from antenv.axon_hooks import get_axon_ntff_profile_hook
        hook = get_axon_ntff_profile_hook()
        neff_dir = tempfile.mkdtemp()
        if hook is None:
            results = run_all()
        else:
            run_all()  # warm: jit trace + NEFF compile before the profiled run
            with hook(neff_dir, [0]):
                results = run_all()
            try:
                import re
                import shutil
                import gauge.profiler
                from concourse._compat import FishPath
                ntffs = sorted(globmod.glob(os.path.join(neff_dir, "*_body*.ntff")))
                times = []
                insts_best = None
                for ntff in ntffs:
                    m = re.search(r"executable(\d+)", os.path.basename(ntff))
                    exe = m.group(1)
                    sub = os.path.join(neff_dir, f"exe{exe}")
                    os.makedirs(sub, exist_ok=True)
                    for fpath in globmod.glob(os.path.join(neff_dir, f"*executable{exe}*")):
                        if os.path.isfile(fpath):
                            shutil.copy(fpath, sub)
                    profile = gauge.profiler.Profile(
                        profile_path=FishPath(sub), kernel_dev_mode=True,
                        profile_on_exit=False, bass_kernel=ncs[0].m,
                        offline_processing=True, fname="*_body*",
                        metadata={"artifacts_path": sub})
                    pr = profile.to_perfetto(model_index=(0,))
                    if pr:
                        times.append(pr[0].exec_time_ns)
                        if pr[0].exec_time_ns == max(times):
                            insts_best = (pr[0].insts, pr[0].trace_path)
                if times:
                    LAST_EXEC_NS = max(times)
                    _LAST_TRACE = insts_best
                    print(f"per-core exec ns: {sorted(times)}", file=sys.stderr)
            except Exception as e:
                print(f"profile post-processing failed: {e!r}", file=sys.stderr)
    else:
        results = run_all()
    return results


def _wscale(W):
    m = float(np.abs(W).max())
    if m <= 0:
        return 1.0
    return float(2.0 ** np.floor(np.log2(200.0 / m)))


def _fp8q(W, s):
    return np.ascontiguousarray(
        np.clip(np.asarray(W, np.float32) * s, -240.0, 240.0).astype(NPFP8))


def kernel(rgns, Unet_segs, region_lens, mi_W1, mi_b1, mi_W2, mi_b2,
           ms_W1, ms_b1, ms_W2, ms_b2, seg_W, seg_b, ln_g, ln_b,
           sc_W, sc_b, f1_W, f1_b):
    _wire_ntff_hook()

    f = lambda a: np.ascontiguousarray(np.asarray(a, dtype=np.float32))
    bf = lambda a: np.ascontiguousarray(np.asarray(a, dtype=np.float32).astype(NPBF16))
    rgns = f(rgns)
    unet = np.asarray(Unet_segs, np.float32).reshape(B, SEG_C, 49).astype(NPBF16)
    lens = np.clip(np.asarray(region_lens).astype(np.int64), 0, R)

    s1, s2, s3, s4 = (_wscale(mi_W1), _wscale(mi_W2), _wscale(sc_W), _wscale(f1_W))
    weights = {
        "mi_W1q": _fp8q(mi_W1, s1), "mi_b1": f(mi_b1).reshape(1, H),
        "mi_W2q": _fp8q(mi_W2, s2), "mi_b2": f(mi_b2).reshape(1, D),
        "ms_W1": bf(ms_W1), "ms_b1": f(ms_b1).reshape(1, H),
        "ms_W2": bf(ms_W2), "ms_b2": f(ms_b2).reshape(1, D),
        "seg_W": bf(seg_W), "seg_b": f(seg_b).reshape(1, D),
        "ln_g": f(ln_g).reshape(1, D), "ln_b": f(ln_b).reshape(1, D),
        "sc_Wq": _fp8q(sc_W, s3), "sc_b": f(sc_b).reshape(1, D),
        "f1_Wq": _fp8q(f1_W, s4), "f1_W": bf(f1_W), "f1_b": f(f1_b).reshape(1, D),
    }

    # balanced batch assignment: 128 batches per core, equalize token counts
    order = np.argsort(-lens, kind="stable")
    loads = np.zeros(NCORES, dtype=np.int64)
    counts = np.zeros(NCORES, dtype=np.int64)
    assign = [[] for _ in range(NCORES)]
    for b in order:
        open_cores = [c for c in range(NCORES) if counts[c] < BC]
        c = min(open_cores, key=lambda c: loads[c])
        assign[c].append(int(b))
        loads[c] += int(lens[b])
        counts[c] += 1
    batches = [np.sort(np.array(a, dtype=np.int64)) for a in assign]

    rflat = rgns.reshape(B * R, D)
    in_maps, plans, vrows = [], [], []
    for c in range(NCORES):
        bl = batches[c]
        lens_c = lens[bl]
        plan = _make_plan(lens_c)
        plans.append(plan)
        rows = np.concatenate([bl[i] * R + np.arange(lens_c[i]) for i in range(BC)])
        vrows.append(rows)
        xc = rflat[rows]
        xbTc = np.ascontiguousarray(xc.astype(NPBF16).T)
        x8Tc = np.ascontiguousarray(
            np.clip(xc, -240.0, 240.0).astype(NPFP8).T)
        in_maps.append(dict(
            x8T=x8Tc,
            xbT=xbTc,
            unet=np.ascontiguousarray(unet[bl]),
            ind=_build_ind_blob(plan[0]),
            **weights,
        ))

    def plan_key(c):
        return (tuple((t["t0"], t["nt"], t["b0"], t["nb"], tuple(t["segs"]))
                      for t in plans[c][0]), (s1, s2, s3, s4))

    keys = [plan_key(c) for c in range(NCORES)]
    uniq = {}
    for c in range(NCORES):
        if keys[c] not in uniq:
            uniq[keys[c]] = None
    with ThreadPoolExecutor(min(8, len(uniq))) as ex:
        built = dict(zip(uniq.keys(),
                         ex.map(lambda k: _get_nc(k, plans[keys.index(k)], (s1, s2, s3, s4)),
                                list(uniq.keys()))))
    ncs = [built[keys[c]] for c in range(NCORES)]

    trace = bool(int(os.environ.get("BASSK_TRACE", "0")))
    results = _run_cores(ncs, in_maps, trace=trace)

    out = np.empty((B * R, D), np.float32)
    out[:] = results[0]["fillv"].reshape(1, D)
    for c in range(NCORES):
        out[vrows[c]] = results[c]["outT"].T.astype(np.float32)
    return out.reshape(B, R, D)


# revision 15
# speedup vs baseline: 2.2490x; 1.1153x over previous
"""Trainium2 Bass kernel for nn_CrossmodalFusion (B=1024, R=36, D=1024).

Data-parallel over batch across 8 NeuronCores with token-level sparsity:
the sigmoid attention mask zeroes every region token with j >=
region_lens[b]; for those tokens the output is exactly relu(f1_b). The
host compacts each core's valid tokens (in (batch, region) order), the
device processes only those (~51%), and the host scatters results back.

The four big per-token GEMMs (mi_W1, mi_W2, sc_W, f1_W) run in fp8-e4m3
DoubleRow mode (2 fp8 contraction planes per PE pass -> half the matmul
instructions of bf16). Weights are host-prescaled by a power of two into
fp8 range; the descale folds into the post-matmul activation `scale`.
All intermediate tensors stay bf16; fp8 copies exist only as matmul
inputs, so quantization error does not chain (CPU-model rel err 1.16e-2
vs the 2e-2 gate). The seg/qw path stays bf16 end-to-end.

Attention weights: masked logits are reduced AND broadcast to 128
partitions in a single ones-matmul on the PE (no DRAM bounce). Initial
loads are spread across the sync/scalar/vector/gpsimd DMA rings so tile
0's mm1 starts as soon as x+W1 land. The output is stored bf16.

Because each core's token plan differs, 8 per-core programs are compiled
(concurrently) and dispatched asynchronously, one per NeuronCore.
"""
import os
import sys
import types
from concurrent.futures import ThreadPoolExecutor
from contextlib import ExitStack

sys.path.insert(0, "/opt/trn_rl_repo")

import numpy as np
import ml_dtypes

import concourse.bass as bass
import concourse.tile as tile
from concourse import bacc, mybir
from concourse.masks import make_identity

F32 = mybir.dt.float32
BF16 = mybir.dt.bfloat16
FP8 = mybir.dt.float8e4

NPBF16 = ml_dtypes.bfloat16
NPFP8 = ml_dtypes.float8_e4m3  # TRN e4m3: max normal 240

B, R, D = 1024, 36, 1024
H = D // 2
SEG_C = 133
NCORES = 8
BC = B // NCORES            # batches per core
KC = D // 128               # 8 feature chunks
KH = H // 128               # 4 hidden chunks

TOKCAP = 512                # tokens per tile (PSUM fp32 bank width)
RSQD = float(1.0 / np.sqrt(D))

LAST_EXEC_NS = None
_LAST_TRACE = None


def _wire_ntff_hook():
    if "antenv.axon_hooks" in sys.modules:
        return
    try:
        import trn_agent_boot.trn_boot as tb
        hook = tb._ntff_profile_via_ctypes("/opt/axon/libaxon_pjrt.so")
    except Exception:
        hook = None
    mod = types.ModuleType("antenv.axon_hooks")
    _h = [hook]
    mod.set_axon_ntff_profile_hook = lambda h: _h.__setitem__(0, h)
    mod.get_axon_ntff_profile_hook = lambda: _h[0]
    sys.modules["antenv.axon_hooks"] = mod


def _make_plan(lens_c):
    """Tile plan for one core from its per-batch valid-token counts."""
    stream = []
    for lb, ln in enumerate(lens_c):
        stream.extend((lb, j) for j in range(int(ln)))
    ntokc = len(stream)
    tiles = []
    t0 = 0
    while t0 < ntokc:
        nt = 0
        b_first = stream[t0][0]
        while t0 + nt < ntokc and nt < TOKCAP:
            lb = stream[t0 + nt][0]
            if lb - b_first + 1 > 128:
                break
            nt += 1
        b_last = stream[t0 + nt - 1][0]
        segs = []
        pos = 0
        while pos < nt:
            lb = stream[t0 + pos][0]
            end = pos
            while end < nt and stream[t0 + end][0] == lb:
                end += 1
            segs.append((lb - b_first, pos, end))
            pos = end
        tiles.append(dict(t0=t0, nt=nt, b0=b_first, nb=b_last - b_first + 1, segs=segs))
        t0 += nt
    return tiles, ntokc


def _emit(ctx, tc, plan, scales):
    nc = tc.nc
    AF = mybir.ActivationFunctionType
    ALU = mybir.AluOpType
    DR = mybir.MatmulPerfMode.DoubleRow
    tiles, ntokc = plan
    s1, s2, s3, s4 = scales

    # ---- DRAM I/O -------------------------------------------------------
    x8T = nc.dram_tensor("x8T", [D, ntokc], FP8, kind="ExternalInput").ap()
    xbT = nc.dram_tensor("xbT", [D, ntokc], BF16, kind="ExternalInput").ap()
    unet = nc.dram_tensor("unet", [BC, SEG_C, 49], BF16, kind="ExternalInput").ap()
    ind_sz = sum(t["nb"] * t["nt"] for t in tiles)
    ind_blob = nc.dram_tensor("ind", [ind_sz], BF16, kind="ExternalInput").ap()
    wi = {}
    for name, shape, dt in [
        ("mi_W1q", [D, H], FP8), ("mi_b1", [1, H], F32),
        ("mi_W2q", [H, D], FP8), ("mi_b2", [1, D], F32),
        ("ms_W1", [D, H], BF16), ("ms_b1", [1, H], F32),
        ("ms_W2", [H, D], BF16), ("ms_b2", [1, D], F32),
        ("seg_W", [SEG_C, D], BF16), ("seg_b", [1, D], F32),
        ("ln_g", [1, D], F32), ("ln_b", [1, D], F32),
        ("sc_Wq", [D, D], FP8), ("sc_b", [1, D], F32),
        ("f1_Wq", [D, D], FP8), ("f1_W", [D, D], BF16), ("f1_b", [1, D], F32),
    ]:
        wi[name] = nc.dram_tensor(name, shape, dt, kind="ExternalInput").ap()
    outT = nc.dram_tensor("outT", [D, ntokc], BF16, kind="ExternalOutput").ap()
    fillv = nc.dram_tensor("fillv", [1, D], F32, kind="ExternalOutput").ap()

    qw_scr = nc.dram_tensor("qw_scr", [BC, D], BF16).ap()

    # ---- persistent constants ------------------------------------------
    const = ctx.enter_context(tc.tile_pool(name="const", bufs=1))

    def load_w(eng, name, kchunks, m, dt):
        t = const.tile([128, kchunks, m], dt, tag=f"cw_{name}")
        eng.dma_start(t[:], wi[name].rearrange("(kc p) m -> p kc m", p=128))
        return t

    # sync ring: what mm1/mm2 need first (then x tiles from the main loop)
    W1q = load_w(nc.sync, "mi_W1q", KC, H, FP8)
    W2q = load_w(nc.sync, "mi_W2q", KH, D, FP8)

    def load_col(name, mchunks):
        ap_ = wi[name]
        t = const.tile([128, mchunks], F32, tag=f"cc_{name}")
        src = bass.AP(tensor=ap_.tensor, offset=ap_.offset, ap=[[1, 128], [128, mchunks]])
        nc.gpsimd.dma_start(t[:], src)
        return t

    b_mi1c = load_col("mi_b1", KH)
    b_mi2c = load_col("mi_b2", KC)
    b_scc = load_col("sc_b", KC)
    b_f1c = load_col("f1_b", KC)

    ones_row = const.tile([1, 512], BF16)
    nc.vector.memset(ones_row[:], 1.0)
    ones_bc = const.tile([128, 128], BF16)   # lhsT for reduce+broadcast
    nc.vector.memset(ones_bc[:], 1.0)
    ident_bf = const.tile([128, 128], BF16)
    make_identity(nc, ident_bf)
    eps_t = const.tile([128, 1], F32)
    nc.vector.memset(eps_t[:], 1e-5)

    qT_bf = const.tile([128, KC, BC], BF16)    # feature-major q (lhsT for attn)

    psum = ctx.enter_context(tc.tile_pool(name="psum", bufs=1, space="PSUM"))

    # ============================ main loop ==============================
    xp8 = ctx.enter_context(tc.tile_pool(name="xp8", bufs=2))
    xpb = ctx.enter_context(tc.tile_pool(name="xpb", bufs=2))
    hp = ctx.enter_context(tc.tile_pool(name="hp", bufs=2))
    rp = ctx.enter_context(tc.tile_pool(name="rp", bufs=2))
    wcp = ctx.enter_context(tc.tile_pool(name="wcp", bufs=2))
    wc8p = ctx.enter_context(tc.tile_pool(name="wc8p", bufs=2))
    scp = ctx.enter_context(tc.tile_pool(name="scp", bufs=3))
    z8p = ctx.enter_context(tc.tile_pool(name="z8p", bufs=2))
    op = ctx.enter_context(tc.tile_pool(name="op", bufs=2))
    sp = ctx.enter_context(tc.tile_pool(name="sp", bufs=2))
    qwp = ctx.enter_context(tc.tile_pool(name="qwp", bufs=max(2, len(tiles))))

    x8T_r = x8T.rearrange("(kc p) t -> p kc t", p=128)
    xbT_r = xbT.rearrange("(kc p) t -> p kc t", p=128)
    outT_r = outT.rearrange("(kc p) t -> p kc t", p=128)

    # Three-stage software pipeline.
    #   A(t): loads + mm1 + mm2 (residual) -> r
    #   C(t): attention + sigmoid + wc (+fp8 pair-copies)
    #   B(t): mm3 + z + mm4 + store
    # Emission: A(0) A(1) A(2), q-stage, then per j: C(j), A(j+3), B(j).
    # The PE's in-order queue thus always holds ~2 stages of independent
    # matmul work to cover scalar/vector latencies, and tile 0's mm1 starts
    # right after its DMA instead of behind the whole q-stage chain.
    ind_offs = []
    off = 0
    for tl in tiles:
        ind_offs.append(off)
        off += tl["nb"] * tl["nt"]
    state = {}

    def emit_A(ti):
        tl = tiles[ti]
        t0, nt, b0, nb = tl["t0"], tl["nt"], tl["b0"], tl["nb"]
        x8 = xp8.tile([128, KC, TOKCAP], FP8, tag="x8")
        nc.sync.dma_start(x8[:, :, 0:nt], x8T_r[:, :, t0:t0 + nt])
        xb = xpb.tile([128, KC, TOKCAP], BF16, tag="xb")
        nc.sync.dma_start(xb[:, :, 0:nt], xbT_r[:, :, t0:t0 + nt])
        ind = sp.tile([nb, TOKCAP], BF16, tag="ind", bufs=4)
        nc.gpsimd.dma_start(ind[:, 0:nt], bass.AP(tensor=ind_blob.tensor,
                                                  offset=ind_blob.offset + ind_offs[ti],
                                                  ap=[[nt, nb], [1, nt]]))
        qw_loc = qwp.tile([nb, D], BF16, tag="qwloc", bufs=4)
        nc.gpsimd.dma_start(qw_loc[:], qw_scr[b0:b0 + nb, :])

        # mm1 (fp8 DR): h1 = relu((x @ W1q)/s1 + b1)
        h1 = hp.tile([128, KH, TOKCAP], FP8, tag="h1")
        for mc in range(KH):
            sl = slice(mc * 128, (mc + 1) * 128)
            ps = psum.tile([128, TOKCAP], F32, tag="mmps", bufs=4)
            for k2 in range(KC // 2):
                nc.tensor.matmul(ps[:, 0:nt], W1q[:, 2 * k2:2 * k2 + 2, sl],
                                 x8[:, 2 * k2:2 * k2 + 2, 0:nt],
                                 start=(k2 == 0), stop=(k2 == KC // 2 - 1),
                                 perf_mode=DR)
            nc.scalar.activation(h1[:, mc, 0:nt], ps[:, 0:nt], AF.Relu,
                                 bias=b_mi1c[:, mc:mc + 1], scale=float(1.0 / s1))

        # mm2 (fp8 DR): r = (h1 @ W2q)/s2 + b2 + x
        r_bf = rp.tile([128, KC, TOKCAP], BF16, tag="r", bufs=3)
        for mc in range(KC):
            sl = slice(mc * 128, (mc + 1) * 128)
            ps = psum.tile([128, TOKCAP], F32, tag="mmps", bufs=4)
            for k2 in range(KH // 2):
                nc.tensor.matmul(ps[:, 0:nt], W2q[:, 2 * k2:2 * k2 + 2, sl],
                                 h1[:, 2 * k2:2 * k2 + 2, 0:nt],
                                 start=(k2 == 0), stop=(k2 == KH // 2 - 1),
                                 perf_mode=DR)
            tmp = sp.tile([128, TOKCAP], BF16, tag="mm2tmp")
            nc.vector.tensor_scalar(tmp[:, 0:nt], ps[:, 0:nt], float(1.0 / s2),
                                    b_mi2c[:, mc:mc + 1], op0=ALU.mult, op1=ALU.add)
            nc.vector.tensor_add(r_bf[:, mc, 0:nt], tmp[:, 0:nt], xb[:, mc, 0:nt])
        state[ti] = dict(nt=nt, t0=t0, b0=b0, nb=nb, r_bf=r_bf, ind=ind,
                         qw_loc=qw_loc)

    def emit_C(ti):
        st = state[ti]
        nt, t0, b0, nb = st["nt"], st["t0"], st["b0"], st["nb"]
        r_bf, ind = st["r_bf"], st["ind"]

        # attention: logits, mask, fused reduce+broadcast on PE, sigmoid
        at = psum.tile([nb, TOKCAP], F32, tag="atps", bufs=2)
        for kc in range(KC):
            nc.tensor.matmul(at[:, 0:nt], qT_bf[:, kc, b0:b0 + nb], r_bf[:, kc, 0:nt],
                             start=(kc == 0), stop=(kc == KC - 1))
        masked = sp.tile([nb, TOKCAP], BF16, tag="msk")
        nc.vector.tensor_tensor(masked[:, 0:nt], at[:, 0:nt], ind[:, 0:nt], op=ALU.mult)
        wb_ps = psum.tile([128, TOKCAP], F32, tag="wbps", bufs=1)
        nc.tensor.matmul(wb_ps[:, 0:nt], ones_bc[0:nb, :], masked[:, 0:nt],
                         start=True, stop=True)
        w_bc = sp.tile([128, TOKCAP], BF16, tag="wbc")
        nc.scalar.activation(w_bc[:, 0:nt], wb_ps[:, 0:nt], AF.Sigmoid, scale=RSQD)

        # wc = w * r (bf16 on DVE) and fp8 pair-copies for mm3 (fp8 writes
        # are fast only on ScalarE; pairing halves the instruction count)
        wc_bf = wcp.tile([128, KC, TOKCAP], BF16, tag="wc")
        wc8 = wc8p.tile([128, KC, TOKCAP], FP8, tag="wc8")
        for kc in range(KC):
            nc.vector.tensor_mul(wc_bf[:, kc, 0:nt], r_bf[:, kc, 0:nt], w_bc[:, 0:nt])
            if kc % 2 == 1:
                nc.scalar.copy(wc8[:, kc - 1:kc + 1, 0:nt], wc_bf[:, kc - 1:kc + 1, 0:nt])
        st["wc_bf"] = wc_bf
        st["wc8"] = wc8

    def emit_B(ti):
        st = state.pop(ti)
        nt, t0 = st["nt"], st["t0"]
        wc_bf, wc8, ind, qw_loc = st["wc_bf"], st["wc8"], st["ind"], st["qw_loc"]

        # mm3 (fp8 DR): scaling = tanh((wc @ W3q)/s3 + sc_b); z = wc*scaling
        z8 = z8p.tile([128, KC, TOKCAP], FP8, tag="z8")
        for mc in range(KC):
            sl = slice(mc * 128, (mc + 1) * 128)
            ps = psum.tile([128, TOKCAP], F32, tag="mmps", bufs=4)
            for k2 in range(KC // 2):
                nc.tensor.matmul(ps[:, 0:nt], W3q[:, 2 * k2:2 * k2 + 2, sl],
                                 wc8[:, 2 * k2:2 * k2 + 2, 0:nt],
                                 start=(k2 == 0), stop=(k2 == KC // 2 - 1),
                                 perf_mode=DR)
            sc = scp.tile([128, TOKCAP], BF16, tag="sc", bufs=2)
            nc.scalar.activation(sc[:, 0:nt], ps[:, 0:nt], AF.Tanh,
                                 bias=b_scc[:, mc:mc + 1], scale=float(1.0 / s3))
            if mc % 2 == 0:
                z_bf = scp.tile([128, 2, TOKCAP], BF16, tag="zbf", bufs=2)
            nc.vector.tensor_mul(z_bf[:, mc % 2, 0:nt], wc_bf[:, mc, 0:nt], sc[:, 0:nt])
            if mc % 2 == 1:
                nc.scalar.copy(z8[:, mc - 1:mc + 1, 0:nt], z_bf[:, :, 0:nt])

        # mm4 (fp8 DR + bf16 seg term): out = relu((z@W4q + s4*qw@ind)/s4 + f1_b)
        for half in range(2):
            o_bf = op.tile([128, KC // 2, TOKCAP], BF16, tag="o")
            for hc in range(KC // 2):
                mc = half * (KC // 2) + hc
                sl = slice(mc * 128, (mc + 1) * 128)
                ps = psum.tile([128, TOKCAP], F32, tag="mmps", bufs=4)
                for k2 in range(KC // 2):
                    nc.tensor.matmul(ps[:, 0:nt], W4q[:, 2 * k2:2 * k2 + 2, sl],
                                     z8[:, 2 * k2:2 * k2 + 2, 0:nt],
                                     start=(k2 == 0), stop=False, perf_mode=DR)
                nc.tensor.matmul(ps[:, 0:nt], qw_loc[:, sl], ind[:, 0:nt],
                                 start=False, stop=True)
                nc.scalar.activation(o_bf[:, hc, 0:nt], ps[:, 0:nt], AF.Relu,
                                     bias=b_f1c[:, mc:mc + 1], scale=float(1.0 / s4))
            nc.scalar.dma_start(
                outT_r[:, half * (KC // 2):(half + 1) * (KC // 2), t0:t0 + nt],
                o_bf[:, :, 0:nt])

    for ti in range(min(3, len(tiles))):
        emit_A(ti)

    # ============================ q-stage ================================
    with tc.tile_pool(name="qpool", bufs=1) as qp:
        pooled = qp.tile([BC, SEG_C], F32)
        for h, cs in enumerate((slice(0, 67), slice(67, SEG_C))):
            unet_h = qp.tile([BC, 67, 49], BF16, tag="unet_h", bufs=1)
            n_c = cs.stop - cs.start
            nc.scalar.dma_start(unet_h[:, 0:n_c, :], unet[:, cs, :])
            nc.vector.reduce_sum(pooled[:, cs], unet_h[:, 0:n_c, :],
                                 axis=mybir.AxisListType.X)
        W_seg_a = qp.tile([128, D], BF16)
        nc.scalar.dma_start(W_seg_a[:], wi["seg_W"][0:128, :])
        W_seg_b = qp.tile([5, D], BF16)
        nc.scalar.dma_start(W_seg_b[:], wi["seg_W"][128:SEG_C, :])
        W_ms1 = qp.tile([128, KC, H], BF16)
        nc.gpsimd.dma_start(W_ms1[:], wi["ms_W1"].rearrange("(kc p) m -> p kc m", p=128))
        W_ms2 = qp.tile([128, KH, D], BF16)
        nc.gpsimd.dma_start(W_ms2[:], wi["ms_W2"].rearrange("(kc p) m -> p kc m", p=128))
        f1W_r = wi["f1_W"].rearrange("(kc p) m -> p kc m", p=128)
        # late-needed fp8 weights go on the scalar ring AFTER the q-stage
        # inputs (ring order ~ emission order; these must land before mm3/mm4
        # of tile 0, ~45us in)
        W3q = load_w(nc.scalar, "sc_Wq", KC, D, FP8)
        W4q = load_w(nc.scalar, "f1_Wq", KC, D, FP8)
        b_segr = qp.tile([1, D], BF16)
        nc.gpsimd.dma_start(b_segr[:], wi["seg_b"])
        b_ms1r = qp.tile([1, H], BF16)
        nc.gpsimd.dma_start(b_ms1r[:], wi["ms_b1"])
        b_ms2r = qp.tile([1, D], BF16)
        nc.gpsimd.dma_start(b_ms2r[:], wi["ms_b2"])
        g_bc = qp.tile([128, D], BF16)
        nc.gpsimd.dma_start(g_bc[:], bass.AP(tensor=wi["ln_g"].tensor, offset=wi["ln_g"].offset, ap=[[0, 128], [1, D]]))
        bb_bc = qp.tile([128, D], BF16)
        nc.gpsimd.dma_start(bb_bc[:], bass.AP(tensor=wi["ln_b"].tensor, offset=wi["ln_b"].offset, ap=[[0, 128], [1, D]]))

        # avgpool(7x7): scale, PE-transpose
        pooled_bf = qp.tile([BC, SEG_C], BF16)
        nc.scalar.mul(pooled_bf[:], pooled[:], 1.0 / 49.0)
        pa_ps = psum.tile([128, BC], BF16, tag="tps", bufs=1)
        nc.tensor.transpose(pa_ps[:], pooled_bf[:, 0:128], ident_bf[0:BC, 0:BC])
        pa_bf = qp.tile([128, BC], BF16)
        nc.scalar.copy(pa_bf[:], pa_ps[:])
        pb_ps = psum.tile([5, BC], BF16, tag="tps", bufs=1)
        nc.tensor.transpose(pb_ps[:], pooled_bf[:, 128:SEG_C], ident_bf[0:BC, 0:BC])
        pb_bf = qp.tile([5, BC], BF16)
        nc.scalar.copy(pb_bf[:], pb_ps[:])

        # q1 = relu(pooled @ seg_W + seg_b)   (token-major: BC x D)
        q1 = qp.tile([BC, D], BF16)
        for ng in range(2):
            sl = slice(ng * 512, (ng + 1) * 512)
            ps = psum.tile([BC, 512], F32, tag="mmps", bufs=4)
            nc.tensor.matmul(ps[:], pa_bf[:], W_seg_a[:, sl], start=True, stop=False)
            nc.tensor.matmul(ps[:], pb_bf[:], W_seg_b[:, sl], start=False, stop=False)
            nc.tensor.matmul(ps[:], ones_row[0:1, 0:BC], b_segr[0:1, sl], start=False, stop=True)
            nc.vector.tensor_scalar_max(q1[:, sl], ps[:], 0.0)

        # layernorm over D
        stats = qp.tile([BC, 2, 6], F32)
        for s in range(2):
            nc.vector.bn_stats(stats[:, s, :], q1[:, s * 512:(s + 1) * 512])
        mv = qp.tile([BC, 2], F32)
        nc.vector.bn_aggr(mv[:], stats[:])
        rstd = qp.tile([BC, 1], F32)
        nc.scalar.activation(rstd[:], mv[:, 1:2], AF.Sqrt, bias=eps_t[0:BC, :])
        nc.vector.reciprocal(rstd[:], rstd[:])
        qn = qp.tile([BC, D], BF16)
        nc.vector.tensor_scalar(qn[:], q1[:], mv[:, 0:1], rstd[:],
                                op0=ALU.subtract, op1=ALU.mult)
        nc.vector.tensor_mul(qn[:], qn[:], g_bc[0:BC, :])
        qn_bf = qp.tile([BC, D], BF16)
        nc.vector.tensor_add(qn_bf[:], qn[:], bb_bc[0:BC, :])

        # qnT (feature-major) via PE transposes
        qnT_bf = qp.tile([128, KC, BC], BF16)
        for kc in range(KC):
            pt = psum.tile([128, BC], BF16, tag="tps", bufs=1)
            nc.tensor.transpose(pt[:], qn_bf[:, kc * 128:(kc + 1) * 128], ident_bf[0:BC, 0:BC])
            nc.scalar.copy(qnT_bf[:, kc, :], pt[:])

        # q MLP (feature-major): qm = relu(ms_W1.T @ qnT + b1)
        qmT_bf = qp.tile([128, KH, BC], BF16)
        for mc in range(KH):
            sl = slice(mc * 128, (mc + 1) * 128)
            ps = psum.tile([128, BC], F32, tag="mmps", bufs=4)
            for kc in range(KC):
                nc.tensor.matmul(ps[:], W_ms1[:, kc, sl], qnT_bf[:, kc, :],
                                 start=(kc == 0), stop=False)
            nc.tensor.matmul(ps[:], b_ms1r[0:1, sl], ones_row[0:1, 0:BC],
                             start=False, stop=True)
            nc.scalar.activation(qmT_bf[:, mc, :], ps[:], AF.Relu)
        # q2T = ms_W2.T @ qmT + b2 + qnT   -> qT_bf
        for mc in range(KC):
            sl = slice(mc * 128, (mc + 1) * 128)
            ps = psum.tile([128, BC], F32, tag="mmps", bufs=4)
            for kc in range(KH):
                nc.tensor.matmul(ps[:], W_ms2[:, kc, sl], qmT_bf[:, kc, :],
                                 start=(kc == 0), stop=False)
            nc.tensor.matmul(ps[:], b_ms2r[0:1, sl], ones_row[0:1, 0:BC],
                             start=False, stop=True)
            nc.vector.tensor_add(qT_bf[:, mc, :], ps[:], qnT_bf[:, mc, :])

        # qw = s4 * (q2 @ f1_W)  (token-major bf16, prescaled to match fp8 psum)
        qw_bf = qp.tile([BC, D], BF16)
        for ng in range(2):
            sl = slice(ng * 512, (ng + 1) * 512)
            W_f1h = qp.tile([128, KC, 512], BF16, tag="wf1h", bufs=1)
            nc.scalar.dma_start(W_f1h[:], f1W_r[:, :, sl])
            ps = psum.tile([BC, 512], F32, tag="mmps", bufs=4)
            for kc in range(KC):
                nc.tensor.matmul(ps[:], qT_bf[:, kc, :], W_f1h[:, kc, :],
                                 start=(kc == 0), stop=(kc == KC - 1))
            nc.scalar.activation(qw_bf[:, sl], ps[:], AF.Identity, scale=float(s4))
        nc.sync.dma_start(qw_scr[:, :], qw_bf[:])

        # fill vector for masked tokens: relu(f1_b)
        fb_row = qp.tile([1, D], F32)
        nc.gpsimd.dma_start(fb_row[:], wi["f1_b"])
        fb_out = qp.tile([1, D], F32)
        nc.vector.tensor_scalar_max(fb_out[:], fb_row[:], 0.0)
        nc.sync.dma_start(fillv[:, :], fb_out[:])

    for j in range(len(tiles)):
        emit_C(j)
        if j + 3 < len(tiles):
            emit_A(j + 3)
        emit_B(j)


def _build(plan, scales):
    nc = bacc.Bacc("TRN2", target_bir_lowering=False, debug=False)
    ctx = ExitStack()
    with tile.TileContext(nc) as tc, ctx:
        _emit(ctx, tc, plan, scales)
    nc.compile()
    return nc


_NC_CACHE = {}


def _get_nc(plan_key, plan, scales):
    if plan_key not in _NC_CACHE:
        _NC_CACHE[plan_key] = _build(plan, scales)
    return _NC_CACHE[plan_key]


def _build_ind_blob(tiles):
    sz = sum(t["nb"] * t["nt"] for t in tiles)
    blob = np.zeros(sz, dtype=NPBF16)
    off = 0
    for t in tiles:
        ind = np.zeros((t["nb"], t["nt"]), dtype=NPBF16)
        for row, lo, hi in t["segs"]:
            ind[row, lo:hi] = 1
        blob[off:off + ind.size] = ind.ravel()
        off += ind.size
    return blob


def _run_cores(ncs, in_maps, trace=False):
    """Dispatch one compiled program per core, concurrently."""
    import jax
    from concourse import bass2jax
    from concourse.bass2jax import _bass_exec_p, install_neuronx_cc_hook

    install_neuronx_cc_hook()
    devices = jax.devices()[:NCORES]

    def make_jit(nc):
        in_names, out_names, out_avals, zero_outs = [], [], [], []
        for alloc in nc.m.functions[0].allocations:
            if not isinstance(alloc, mybir.MemoryLocationSet):
                continue
            name = alloc.memorylocations[0].name
            if alloc.kind == "ExternalInput":
                in_names.append(name)
            elif alloc.kind == "ExternalOutput":
                out_names.append(name)
                shape = tuple(alloc.tensor_shape)
                dtype = mybir.dt.np(alloc.dtype)
                out_avals.append(jax.core.ShapedArray(shape, dtype))
                zero_outs.append(np.zeros(shape, dtype))
        n_params = len(in_names)
        all_names = in_names + out_names

        def _body(*args):
            outs = _bass_exec_p.bind(
                *args,
                out_avals=tuple(out_avals),
                in_names=tuple(all_names),
                out_names=tuple(out_names),
                lowering_input_output_aliases=(),
                sim_require_finite=True,
                sim_require_nnan=True,
                nc=nc,
            )
            return tuple(outs)

        donate = tuple(range(n_params, n_params + len(out_names)))
        return (jax.jit(_body, donate_argnums=donate, keep_unused=True),
                in_names, out_names, zero_outs)

    with ThreadPoolExecutor(NCORES) as ex:
        jits = list(ex.map(make_jit, ncs))

    def launch(c):
        jitted, in_names, out_names, zero_outs = jits[c]
        vals = dict(in_maps[c])
        pid = ncs[c].partition_id_tensor
        if pid is not None:
            vals[pid.name] = np.array([[c]], dtype=np.uint32)
        args = [jax.device_put(np.asarray(vals[n]), devices[c]) for n in in_names]
        zz = [jax.device_put(z, devices[c]) for z in zero_outs]
        outs = jitted(*args, *zz)
        return dict(zip(out_names, outs))

    def run_all():
        with ThreadPoolExecutor(NCORES) as ex:
            outs = list(ex.map(launch, range(NCORES)))
        return [{k: np.asarray(v) for k, v in o.items()} for o in outs]

    global LAST_EXEC_NS, _LAST_TRACE
    if trace:
        import glob as globmod
        import tempfile
        from antenv.axon_hooks import get_axon_ntff_profile_hook
        hook = get_axon_ntff_profile_hook()
        neff_dir = tempfile.mkdtemp()
        if hook is None:
            results = run_all()
        else:
            run_all()  # warm: jit trace + NEFF compile before the profiled run
            with hook(neff_dir, [0]):
                results = run_all()
            try:
                import re
                import shutil
                import gauge.profiler
                from concourse._compat import FishPath
                ntffs = sorted(globmod.glob(os.path.join(neff_dir, "*_body*.ntff")))
                times = []
                insts_best = None
                for ntff in ntffs:
                    m = re.search(r"executable(\d+)", os.path.basename(ntff))
                    exe = m.group(1)
                    sub = os.path.join(neff_dir, f"exe{exe}")
                    os.makedirs(sub, exist_ok=True)
                    for fpath in globmod.glob(os.path.join(neff_dir, f"*executable{exe}*")):
                        if os.path.isfile(fpath):
                            shutil.copy(fpath, sub)
                    profile = gauge.profiler.Profile(
                        profile_path=FishPath(sub), kernel_dev_mode=True,
                        profile_on_exit=False, bass_kernel=ncs[0].m,
                        offline_processing=True, fname="*_body*",
                        metadata={"artifacts_path": sub})
                    pr = profile.to_perfetto(model_index=(0,))
                    if pr:
                        times.append(pr[0].exec_time_ns)
                        if pr[0].exec_time_ns == max(times):
                            insts_best = (pr[0].insts, pr[0].trace_path)
                if times:
                    LAST_EXEC_NS = max(times)
                    _LAST_TRACE = insts_best
                    print(f"per-core exec ns: {sorted(times)}", file=sys.stderr)
            except Exception as e:
                print(f"profile post-processing failed: {e!r}", file=sys.stderr)
    else:
        results = run_all()
    return results


def _wscale(W):
    m = float(np.abs(W).max())
    if m <= 0:
        return 1.0
    return float(2.0 ** np.floor(np.log2(200.0 / m)))


def _fp8q(W, s):
    return np.ascontiguousarray(
        np.clip(np.asarray(W, np.float32) * s, -240.0, 240.0).astype(NPFP8))


def kernel(rgns, Unet_segs, region_lens, mi_W1, mi_b1, mi_W2, mi_b2,
           ms_W1, ms_b1, ms_W2, ms_b2, seg_W, seg_b, ln_g, ln_b,
           sc_W, sc_b, f1_W, f1_b):
    _wire_ntff_hook()

    f = lambda a: np.ascontiguousarray(np.asarray(a, dtype=np.float32))
    bf = lambda a: np.ascontiguousarray(np.asarray(a, dtype=np.float32).astype(NPBF16))
    rgns = f(rgns)
    unet = np.asarray(Unet_segs, np.float32).reshape(B, SEG_C, 49).astype(NPBF16)
    lens = np.clip(np.asarray(region_lens).astype(np.int64), 0, R)

    s1, s2, s3, s4 = (_wscale(mi_W1), _wscale(mi_W2), _wscale(sc_W), _wscale(f1_W))
    weights = {
        "mi_W1q": _fp8q(mi_W1, s1), "mi_b1": f(mi_b1).reshape(1, H),
        "mi_W2q": _fp8q(mi_W2, s2), "mi_b2": f(mi_b2).reshape(1, D),
        "ms_W1": bf(ms_W1), "ms_b1": f(ms_b1).reshape(1, H),
        "ms_W2": bf(ms_W2), "ms_b2": f(ms_b2).reshape(1, D),
        "seg_W": bf(seg_W), "seg_b": f(seg_b).reshape(1, D),
        "ln_g": f(ln_g).reshape(1, D), "ln_b": f(ln_b).reshape(1, D),
        "sc_Wq": _fp8q(sc_W, s3), "sc_b": f(sc_b).reshape(1, D),
        "f1_Wq": _fp8q(f1_W, s4), "f1_W": bf(f1_W), "f1_b": f(f1_b).reshape(1, D),
    }

    # balanced batch assignment: 128 batches per core, equalize token counts
    order = np.argsort(-lens, kind="stable")
    loads = np.zeros(NCORES, dtype=np.int64)
    counts = np.zeros(NCORES, dtype=np.int64)
    assign = [[] for _ in range(NCORES)]
    for b in order:
        open_cores = [c for c in range(NCORES) if counts[c] < BC]
        c = min(open_cores, key=lambda c: loads[c])
        assign[c].append(int(b))
        loads[c] += int(lens[b])
        counts[c] += 1
    batches = [np.sort(np.array(a, dtype=np.int64)) for a in assign]

    rflat = rgns.reshape(B * R, D)
    in_maps, plans, vrows = [], [], []
    for c in range(NCORES):
        bl = batches[c]
        lens_c = lens[bl]
        plan = _make_plan(lens_c)
        plans.append(plan)
        rows = np.concatenate([bl[i] * R + np.arange(lens_c[i]) for i in range(BC)])
        vrows.append(rows)
        xc = rflat[rows]
        xbTc = np.ascontiguousarray(xc.astype(NPBF16).T)
        x8Tc = np.ascontiguousarray(
            np.clip(xc, -240.0, 240.0).astype(NPFP8).T)
        in_maps.append(dict(
            x8T=x8Tc,
            xbT=xbTc,
            unet=np.ascontiguousarray(unet[bl]),
            ind=_build_ind_blob(plan[0]),
            **weights,
        ))

    def plan_key(c):
        return (tuple((t["t0"], t["nt"], t["b0"], t["nb"], tuple(t["segs"]))
                      for t in plans[c][0]), (s1, s2, s3, s4))

    keys = [plan_key(c) for c in range(NCORES)]
    uniq = {}
    for c in range(NCORES):
        if keys[c] not in uniq:
            uniq[keys[c]] = None
    with ThreadPoolExecutor(min(8, len(uniq))) as ex:
        built = dict(zip(uniq.keys(),
                         ex.map(lambda k: _get_nc(k, plans[keys.index(k)], (s1, s2, s3, s4)),
                                list(uniq.keys()))))
    ncs = [built[keys[c]] for c in range(NCORES)]

    trace = bool(int(os.environ.get("BASSK_TRACE", "0")))
    results = _run_cores(ncs, in_maps, trace=trace)

    out = np.empty((B * R, D), np.float32)
    out[:] = results[0]["fillv"].reshape(1, D)
    for c in range(NCORES):
        out[vrows[c]] = results[c]["outT"].T.astype(np.float32)
    return out.reshape(B, R, D)


# revision 17
# speedup vs baseline: 2.3616x; 1.0501x over previous
"""Trainium2 Bass kernel for nn_CrossmodalFusion (B=1024, R=36, D=1024).

Data-parallel over batch across 8 NeuronCores with token-level sparsity:
the sigmoid attention mask zeroes every region token with j >=
region_lens[b]; for those tokens the output is exactly relu(f1_b). The
host compacts each core's valid tokens (in (batch, region) order), the
device processes only those (~51%), and the host scatters results back.

The four big per-token GEMMs (mi_W1, mi_W2, sc_W, f1_W) run in fp8-e4m3
DoubleRow mode (2 fp8 contraction planes per PE pass -> half the matmul
instructions of bf16). Weights are host-prescaled by a power of two into
fp8 range; the descale folds into the post-matmul activation `scale`.
All intermediate tensors stay bf16; fp8 copies exist only as matmul
inputs, so quantization error does not chain (CPU-model rel err 1.16e-2
vs the 2e-2 gate). The seg/qw path stays bf16 end-to-end.

Attention weights: masked logits are reduced AND broadcast to 128
partitions in a single ones-matmul on the PE (no DRAM bounce). Initial
loads are spread across the sync/scalar/vector/gpsimd DMA rings so tile
0's mm1 starts as soon as x+W1 land. The output is stored bf16.

Because each core's token plan differs, 8 per-core programs are compiled
(concurrently) and dispatched asynchronously, one per NeuronCore.
"""
import os
import sys
import types
from concurrent.futures import ThreadPoolExecutor
from contextlib import ExitStack

sys.path.insert(0, "/opt/trn_rl_repo")

import numpy as np
import ml_dtypes

import concourse.bass as bass
import concourse.tile as tile
from concourse import bacc, mybir
from concourse.masks import make_identity

F32 = mybir.dt.float32
BF16 = mybir.dt.bfloat16
FP8 = mybir.dt.float8e4

NPBF16 = ml_dtypes.bfloat16
NPFP8 = ml_dtypes.float8_e4m3  # TRN e4m3: max normal 240

B, R, D = 1024, 36, 1024
H = D // 2
SEG_C = 133
NCORES = 8
BC = B // NCORES            # batches per core
KC = D // 128               # 8 feature chunks
KH = H // 128               # 4 hidden chunks

TOKCAP = 512                # tokens per tile (PSUM fp32 bank width)
RSQD = float(1.0 / np.sqrt(D))

LAST_EXEC_NS = None
_LAST_TRACE = None


def _wire_ntff_hook():
    if "antenv.axon_hooks" in sys.modules:
        return
    try:
        import trn_agent_boot.trn_boot as tb
        hook = tb._ntff_profile_via_ctypes("/opt/axon/libaxon_pjrt.so")
    except Exception:
        hook = None
    mod = types.ModuleType("antenv.axon_hooks")
    _h = [hook]
    mod.set_axon_ntff_profile_hook = lambda h: _h.__setitem__(0, h)
    mod.get_axon_ntff_profile_hook = lambda: _h[0]
    sys.modules["antenv.axon_hooks"] = mod


def _make_plan(lens_c):
    """Tile plan for one core from its per-batch valid-token counts."""
    stream = []
    for lb, ln in enumerate(lens_c):
        stream.extend((lb, j) for j in range(int(ln)))
    ntokc = len(stream)
    tiles = []
    t0 = 0
    while t0 < ntokc:
        nt = 0
        b_first = stream[t0][0]
        while t0 + nt < ntokc and nt < TOKCAP:
            lb = stream[t0 + nt][0]
            if lb - b_first + 1 > 128:
                break
            nt += 1
        b_last = stream[t0 + nt - 1][0]
        segs = []
        pos = 0
        while pos < nt:
            lb = stream[t0 + pos][0]
            end = pos
            while end < nt and stream[t0 + end][0] == lb:
                end += 1
            segs.append((lb - b_first, pos, end))
            pos = end
        tiles.append(dict(t0=t0, nt=nt, b0=b_first, nb=b_last - b_first + 1, segs=segs))
        t0 += nt
    return tiles, ntokc


def _emit(ctx, tc, plan, scales):
    nc = tc.nc
    AF = mybir.ActivationFunctionType
    ALU = mybir.AluOpType
    DR = mybir.MatmulPerfMode.DoubleRow
    tiles, ntokc = plan
    s1, s2, s3, s4 = scales

    # ---- DRAM I/O -------------------------------------------------------
    # x/out travel as tile-major blobs: per tile a [128, KC*nt] block that is
    # contiguous per partition (128 DMA descriptors instead of 1024)
    blob_cols = sum(KC * t["nt"] for t in tiles)
    x8B = nc.dram_tensor("x8B", [128, blob_cols], FP8, kind="ExternalInput").ap()
    xbB = nc.dram_tensor("xbB", [128, blob_cols], BF16, kind="ExternalInput").ap()
    unet = nc.dram_tensor("unet", [BC, SEG_C, 49], BF16, kind="ExternalInput").ap()
    ind_sz = sum(t["nb"] * t["nt"] for t in tiles)
    ind_blob = nc.dram_tensor("ind", [ind_sz], BF16, kind="ExternalInput").ap()
    wi = {}
    for name, shape, dt in [
        ("mi_W1q", [128, KC * H], FP8), ("mi_b1", [1, H], F32),
        ("mi_W2q", [128, KH * D], FP8), ("mi_b2", [1, D], F32),
        ("ms_W1", [128, KC * H], BF16), ("ms_b1", [1, H], F32),
        ("ms_W2", [128, KH * D], BF16), ("ms_b2", [1, D], F32),
        ("seg_W", [SEG_C, D], BF16), ("seg_b", [1, D], F32),
        ("ln_g", [1, D], F32), ("ln_b", [1, D], F32),
        ("sc_Wq", [128, KC * D], FP8), ("sc_b", [1, D], F32),
        ("f1_Wq", [128, KC * D], FP8), ("f1_W", [D, D], BF16), ("f1_b", [1, D], F32),
    ]:
        wi[name] = nc.dram_tensor(name, shape, dt, kind="ExternalInput").ap()
    outB = nc.dram_tensor("outB", [128, blob_cols], BF16, kind="ExternalOutput").ap()
    fillv = nc.dram_tensor("fillv", [1, D], F32, kind="ExternalOutput").ap()

    qw_scr = nc.dram_tensor("qw_scr", [BC, D], BF16).ap()

    # ---- persistent constants ------------------------------------------
    const = ctx.enter_context(tc.tile_pool(name="const", bufs=1))

    def load_w(eng, name, kchunks, m, dt):
        # host pre-arranges weights to [128, kchunks*m], contiguous/partition
        t = const.tile([128, kchunks, m], dt, tag=f"cw_{name}")
        ap_ = wi[name]
        src_ = bass.AP(tensor=ap_.tensor, offset=ap_.offset,
                       ap=[[kchunks * m, 128], [m, kchunks], [1, m]])
        eng.dma_start(t[:], src_)
        return t

    # sync ring: what mm1/mm2 need first (then x tiles from the main loop)
    W1q = load_w(nc.sync, "mi_W1q", KC, H, FP8)
    W2q = load_w(nc.sync, "mi_W2q", KH, D, FP8)

    def load_col(name, mchunks):
        ap_ = wi[name]
        t = const.tile([128, mchunks], F32, tag=f"cc_{name}")
        src = bass.AP(tensor=ap_.tensor, offset=ap_.offset, ap=[[1, 128], [128, mchunks]])
        nc.gpsimd.dma_start(t[:], src)
        return t

    b_mi1c = load_col("mi_b1", KH)
    b_mi2c = load_col("mi_b2", KC)
    b_scc = load_col("sc_b", KC)
    b_f1c = load_col("f1_b", KC)

    ones_row = const.tile([1, 512], BF16)
    nc.vector.memset(ones_row[:], 1.0)
    ones_bc = const.tile([128, 128], BF16)   # lhsT for reduce+broadcast
    nc.vector.memset(ones_bc[:], 1.0)
    ident_bf = const.tile([128, 128], BF16)
    make_identity(nc, ident_bf)
    eps_t = const.tile([128, 1], F32)
    nc.vector.memset(eps_t[:], 1e-5)

    qT_bf = const.tile([128, KC, BC], BF16)    # feature-major q (lhsT for attn)

    psum = ctx.enter_context(tc.tile_pool(name="psum", bufs=1, space="PSUM"))

    # ============================ main loop ==============================
    xp8 = ctx.enter_context(tc.tile_pool(name="xp8", bufs=2))
    xpb = ctx.enter_context(tc.tile_pool(name="xpb", bufs=2))
    hp = ctx.enter_context(tc.tile_pool(name="hp", bufs=2))
    rp = ctx.enter_context(tc.tile_pool(name="rp", bufs=2))
    wcp = ctx.enter_context(tc.tile_pool(name="wcp", bufs=2))
    wc8p = ctx.enter_context(tc.tile_pool(name="wc8p", bufs=2))
    scp = ctx.enter_context(tc.tile_pool(name="scp", bufs=3))
    z8p = ctx.enter_context(tc.tile_pool(name="z8p", bufs=2))
    op = ctx.enter_context(tc.tile_pool(name="op", bufs=2))
    sp = ctx.enter_context(tc.tile_pool(name="sp", bufs=2))
    qwp = ctx.enter_context(tc.tile_pool(name="qwp", bufs=max(2, len(tiles))))

    blob_offs = []
    boff = 0
    for t in tiles:
        blob_offs.append(boff)
        boff += KC * t["nt"]

    # Three-stage software pipeline.
    #   A(t): loads + mm1 + mm2 (residual) -> r
    #   C(t): attention + sigmoid + wc (+fp8 pair-copies)
    #   B(t): mm3 + z + mm4 + store
    # Emission: A(0) A(1) A(2), q-stage, then per j: C(j), A(j+3), B(j).
    # The PE's in-order queue thus always holds ~2 stages of independent
    # matmul work to cover scalar/vector latencies, and tile 0's mm1 starts
    # right after its DMA instead of behind the whole q-stage chain.
    ind_offs = []
    off = 0
    for tl in tiles:
        ind_offs.append(off)
        off += tl["nb"] * tl["nt"]
    state = {}

    def emit_A(ti):
        tl = tiles[ti]
        t0, nt, b0, nb = tl["t0"], tl["nt"], tl["b0"], tl["nb"]
        bo = blob_offs[ti]
        x8 = xp8.tile([128, KC, TOKCAP], FP8, tag="x8")
        nc.sync.dma_start(x8[:, :, 0:nt], bass.AP(
            tensor=x8B.tensor, offset=x8B.offset + bo,
            ap=[[blob_cols, 128], [nt, KC], [1, nt]]))
        xb = xpb.tile([128, KC, TOKCAP], BF16, tag="xb")
        nc.sync.dma_start(xb[:, :, 0:nt], bass.AP(
            tensor=xbB.tensor, offset=xbB.offset + bo,
            ap=[[blob_cols, 128], [nt, KC], [1, nt]]))
        ind = sp.tile([nb, TOKCAP], BF16, tag="ind", bufs=4)
        nc.gpsimd.dma_start(ind[:, 0:nt], bass.AP(tensor=ind_blob.tensor,
                                                  offset=ind_blob.offset + ind_offs[ti],
                                                  ap=[[nt, nb], [1, nt]]))
        qw_loc = qwp.tile([nb, D], BF16, tag="qwloc", bufs=4)
        nc.gpsimd.dma_start(qw_loc[:], qw_scr[b0:b0 + nb, :])

        # mm1 (fp8 DR): h1 = relu((x @ W1q)/s1 + b1)
        h1 = hp.tile([128, KH, TOKCAP], FP8, tag="h1")
        for mc in range(KH):
            sl = slice(mc * 128, (mc + 1) * 128)
            ps = psum.tile([128, TOKCAP], F32, tag="mmps", bufs=4)
            for k2 in range(KC // 2):
                nc.tensor.matmul(ps[:, 0:nt], W1q[:, 2 * k2:2 * k2 + 2, sl],
                                 x8[:, 2 * k2:2 * k2 + 2, 0:nt],
                                 start=(k2 == 0), stop=(k2 == KC // 2 - 1),
                                 perf_mode=DR)
            nc.scalar.activation(h1[:, mc, 0:nt], ps[:, 0:nt], AF.Relu,
                                 bias=b_mi1c[:, mc:mc + 1], scale=float(1.0 / s1))

        # mm2 (fp8 DR): r = (h1 @ W2q)/s2 + b2 + x
        r_bf = rp.tile([128, KC, TOKCAP], BF16, tag="r", bufs=3)
        for mc in range(KC):
            sl = slice(mc * 128, (mc + 1) * 128)
            ps = psum.tile([128, TOKCAP], F32, tag="mmps", bufs=4)
            for k2 in range(KH // 2):
                nc.tensor.matmul(ps[:, 0:nt], W2q[:, 2 * k2:2 * k2 + 2, sl],
                                 h1[:, 2 * k2:2 * k2 + 2, 0:nt],
                                 start=(k2 == 0), stop=(k2 == KH // 2 - 1),
                                 perf_mode=DR)
            tmp = sp.tile([128, TOKCAP], BF16, tag="mm2tmp")
            nc.vector.tensor_scalar(tmp[:, 0:nt], ps[:, 0:nt], float(1.0 / s2),
                                    b_mi2c[:, mc:mc + 1], op0=ALU.mult, op1=ALU.add)
            nc.vector.tensor_add(r_bf[:, mc, 0:nt], tmp[:, 0:nt], xb[:, mc, 0:nt])
        state[ti] = dict(nt=nt, t0=t0, b0=b0, nb=nb, r_bf=r_bf, ind=ind,
                         qw_loc=qw_loc)

    def emit_C(ti):
        st = state[ti]
        nt, t0, b0, nb = st["nt"], st["t0"], st["b0"], st["nb"]
        r_bf, ind = st["r_bf"], st["ind"]

        # attention: logits, mask, fused reduce+broadcast on PE, sigmoid
        at = psum.tile([nb, TOKCAP], F32, tag="atps", bufs=2)
        for kc in range(KC):
            nc.tensor.matmul(at[:, 0:nt], qT_bf[:, kc, b0:b0 + nb], r_bf[:, kc, 0:nt],
                             start=(kc == 0), stop=(kc == KC - 1))
        masked = sp.tile([nb, TOKCAP], BF16, tag="msk")
        nc.vector.tensor_tensor(masked[:, 0:nt], at[:, 0:nt], ind[:, 0:nt], op=ALU.mult)
        wb_ps = psum.tile([128, TOKCAP], F32, tag="wbps", bufs=1)
        nc.tensor.matmul(wb_ps[:, 0:nt], ones_bc[0:nb, :], masked[:, 0:nt],
                         start=True, stop=True)
        w_bc = sp.tile([128, TOKCAP], BF16, tag="wbc")
        nc.scalar.activation(w_bc[:, 0:nt], wb_ps[:, 0:nt], AF.Sigmoid, scale=RSQD)

        # wc = w * r (bf16 on DVE) and fp8 pair-copies for mm3 (fp8 writes
        # are fast only on ScalarE; pairing halves the instruction count)
        wc_bf = wcp.tile([128, KC, TOKCAP], BF16, tag="wc")
        wc8 = wc8p.tile([128, KC, TOKCAP], FP8, tag="wc8")
        for kc in range(KC):
            nc.vector.tensor_mul(wc_bf[:, kc, 0:nt], r_bf[:, kc, 0:nt], w_bc[:, 0:nt])
            if kc % 2 == 1:
                nc.scalar.copy(wc8[:, kc - 1:kc + 1, 0:nt], wc_bf[:, kc - 1:kc + 1, 0:nt])
        st["wc_bf"] = wc_bf
        st["wc8"] = wc8

    def emit_B(ti):
        st = state.pop(ti)
        nt, t0 = st["nt"], st["t0"]
        wc_bf, wc8, ind, qw_loc = st["wc_bf"], st["wc8"], st["ind"], st["qw_loc"]

        # mm3 (fp8 DR): scaling = tanh((wc @ W3q)/s3 + sc_b); z = wc*scaling
        z8 = z8p.tile([128, KC, TOKCAP], FP8, tag="z8")
        for mc in range(KC):
            sl = slice(mc * 128, (mc + 1) * 128)
            ps = psum.tile([128, TOKCAP], F32, tag="mmps", bufs=4)
            for k2 in range(KC // 2):
                nc.tensor.matmul(ps[:, 0:nt], W3q[:, 2 * k2:2 * k2 + 2, sl],
                                 wc8[:, 2 * k2:2 * k2 + 2, 0:nt],
                                 start=(k2 == 0), stop=(k2 == KC // 2 - 1),
                                 perf_mode=DR)
            sc = scp.tile([128, TOKCAP], BF16, tag="sc", bufs=2)
            nc.scalar.activation(sc[:, 0:nt], ps[:, 0:nt], AF.Tanh,
                                 bias=b_scc[:, mc:mc + 1], scale=float(1.0 / s3))
            if mc % 2 == 0:
                z_bf = scp.tile([128, 2, TOKCAP], BF16, tag="zbf", bufs=2)
            nc.vector.tensor_mul(z_bf[:, mc % 2, 0:nt], wc_bf[:, mc, 0:nt], sc[:, 0:nt])
            if mc % 2 == 1:
                nc.scalar.copy(z8[:, mc - 1:mc + 1, 0:nt], z_bf[:, :, 0:nt])

        # mm4 (fp8 DR + bf16 seg term): out = relu((z@W4q + s4*qw@ind)/s4 + f1_b)
        for half in range(2):
            o_bf = op.tile([128, KC // 2, TOKCAP], BF16, tag="o")
            for hc in range(KC // 2):
                mc = half * (KC // 2) + hc
                sl = slice(mc * 128, (mc + 1) * 128)
                ps = psum.tile([128, TOKCAP], F32, tag="mmps", bufs=4)
                for k2 in range(KC // 2):
                    nc.tensor.matmul(ps[:, 0:nt], W4q[:, 2 * k2:2 * k2 + 2, sl],
                                     z8[:, 2 * k2:2 * k2 + 2, 0:nt],
                                     start=(k2 == 0), stop=False, perf_mode=DR)
                nc.tensor.matmul(ps[:, 0:nt], qw_loc[:, sl], ind[:, 0:nt],
                                 start=False, stop=True)
                nc.scalar.activation(o_bf[:, hc, 0:nt], ps[:, 0:nt], AF.Relu,
                                     bias=b_f1c[:, mc:mc + 1], scale=float(1.0 / s4))
            nc.scalar.dma_start(bass.AP(
                tensor=outB.tensor,
                offset=outB.offset + blob_offs[ti] + half * (KC // 2) * nt,
                ap=[[blob_cols, 128], [nt, KC // 2], [1, nt]]),
                o_bf[:, :, 0:nt])

    for ti in range(min(3, len(tiles))):
        emit_A(ti)

    # ============================ q-stage ================================
    with tc.tile_pool(name="qpool", bufs=1) as qp:
        pooled = qp.tile([BC, SEG_C], F32)
        for h, cs in enumerate((slice(0, 67), slice(67, SEG_C))):
            unet_h = qp.tile([BC, 67, 49], BF16, tag="unet_h", bufs=1)
            n_c = cs.stop - cs.start
            nc.scalar.dma_start(unet_h[:, 0:n_c, :], unet[:, cs, :])
            nc.vector.reduce_sum(pooled[:, cs], unet_h[:, 0:n_c, :],
                                 axis=mybir.AxisListType.X)
        W_seg_a = qp.tile([128, D], BF16)
        nc.scalar.dma_start(W_seg_a[:], wi["seg_W"][0:128, :])
        W_seg_b = qp.tile([5, D], BF16)
        nc.scalar.dma_start(W_seg_b[:], wi["seg_W"][128:SEG_C, :])
        W_ms1 = qp.tile([128, KC, H], BF16, tag="qms1")
        ap1 = wi["ms_W1"]
        nc.gpsimd.dma_start(W_ms1[:], bass.AP(tensor=ap1.tensor, offset=ap1.offset,
                                              ap=[[KC * H, 128], [H, KC], [1, H]]))
        W_ms2 = qp.tile([128, KH, D], BF16, tag="qms2")
        ap2 = wi["ms_W2"]
        nc.gpsimd.dma_start(W_ms2[:], bass.AP(tensor=ap2.tensor, offset=ap2.offset,
                                              ap=[[KH * D, 128], [D, KH], [1, D]]))
        f1W_r = wi["f1_W"].rearrange("(kc p) m -> p kc m", p=128)
        # late-needed fp8 weights go on the scalar ring AFTER the q-stage
        # inputs (ring order ~ emission order; these must land before mm3/mm4
        # of tile 0, ~45us in)
        W3q = load_w(nc.scalar, "sc_Wq", KC, D, FP8)
        W4q = load_w(nc.scalar, "f1_Wq", KC, D, FP8)
        b_segr = qp.tile([1, D], BF16)
        nc.gpsimd.dma_start(b_segr[:], wi["seg_b"])
        b_ms1r = qp.tile([1, H], BF16)
        nc.gpsimd.dma_start(b_ms1r[:], wi["ms_b1"])
        b_ms2r = qp.tile([1, D], BF16)
        nc.gpsimd.dma_start(b_ms2r[:], wi["ms_b2"])
        g_bc = qp.tile([128, D], BF16)
        nc.gpsimd.dma_start(g_bc[:], bass.AP(tensor=wi["ln_g"].tensor, offset=wi["ln_g"].offset, ap=[[0, 128], [1, D]]))
        bb_bc = qp.tile([128, D], BF16)
        nc.gpsimd.dma_start(bb_bc[:], bass.AP(tensor=wi["ln_b"].tensor, offset=wi["ln_b"].offset, ap=[[0, 128], [1, D]]))

        # avgpool(7x7): scale, PE-transpose
        pooled_bf = qp.tile([BC, SEG_C], BF16)
        nc.scalar.mul(pooled_bf[:], pooled[:], 1.0 / 49.0)
        pa_ps = psum.tile([128, BC], BF16, tag="tps", bufs=1)
        nc.tensor.transpose(pa_ps[:], pooled_bf[:, 0:128], ident_bf[0:BC, 0:BC])
        pa_bf = qp.tile([128, BC], BF16)
        nc.scalar.copy(pa_bf[:], pa_ps[:])
        pb_ps = psum.tile([5, BC], BF16, tag="tps", bufs=1)
        nc.tensor.transpose(pb_ps[:], pooled_bf[:, 128:SEG_C], ident_bf[0:BC, 0:BC])
        pb_bf = qp.tile([5, BC], BF16)
        nc.scalar.copy(pb_bf[:], pb_ps[:])

        # q1 = relu(pooled @ seg_W + seg_b)   (token-major: BC x D)
        q1 = qp.tile([BC, D], BF16)
        for ng in range(2):
            sl = slice(ng * 512, (ng + 1) * 512)
            ps = psum.tile([BC, 512], F32, tag="mmps", bufs=4)
            nc.tensor.matmul(ps[:], pa_bf[:], W_seg_a[:, sl], start=True, stop=False)
            nc.tensor.matmul(ps[:], pb_bf[:], W_seg_b[:, sl], start=False, stop=False)
            nc.tensor.matmul(ps[:], ones_row[0:1, 0:BC], b_segr[0:1, sl], start=False, stop=True)
            nc.vector.tensor_scalar_max(q1[:, sl], ps[:], 0.0)

        # layernorm over D
        stats = qp.tile([BC, 2, 6], F32)
        for s in range(2):
            nc.vector.bn_stats(stats[:, s, :], q1[:, s * 512:(s + 1) * 512])
        mv = qp.tile([BC, 2], F32)
        nc.vector.bn_aggr(mv[:], stats[:])
        rstd = qp.tile([BC, 1], F32)
        nc.scalar.activation(rstd[:], mv[:, 1:2], AF.Sqrt, bias=eps_t[0:BC, :])
        nc.vector.reciprocal(rstd[:], rstd[:])
        qn = qp.tile([BC, D], BF16)
        nc.vector.tensor_scalar(qn[:], q1[:], mv[:, 0:1], rstd[:],
                                op0=ALU.subtract, op1=ALU.mult)
        nc.vector.tensor_mul(qn[:], qn[:], g_bc[0:BC, :])
        qn_bf = qp.tile([BC, D], BF16)
        nc.vector.tensor_add(qn_bf[:], qn[:], bb_bc[0:BC, :])

        # qnT (feature-major) via PE transposes
        qnT_bf = qp.tile([128, KC, BC], BF16)
        for kc in range(KC):
            pt = psum.tile([128, BC], BF16, tag="tps", bufs=1)
            nc.tensor.transpose(pt[:], qn_bf[:, kc * 128:(kc + 1) * 128], ident_bf[0:BC, 0:BC])
            nc.scalar.copy(qnT_bf[:, kc, :], pt[:])

        # q MLP (feature-major): qm = relu(ms_W1.T @ qnT + b1)
        qmT_bf = qp.tile([128, KH, BC], BF16)
        for mc in range(KH):
            sl = slice(mc * 128, (mc + 1) * 128)
            ps = psum.tile([128, BC], F32, tag="mmps", bufs=4)
            for kc in range(KC):
                nc.tensor.matmul(ps[:], W_ms1[:, kc, sl], qnT_bf[:, kc, :],
                                 start=(kc == 0), stop=False)
            nc.tensor.matmul(ps[:], b_ms1r[0:1, sl], ones_row[0:1, 0:BC],
                             start=False, stop=True)
            nc.scalar.activation(qmT_bf[:, mc, :], ps[:], AF.Relu)
        # q2T = ms_W2.T @ qmT + b2 + qnT   -> qT_bf
        for mc in range(KC):
            sl = slice(mc * 128, (mc + 1) * 128)
            ps = psum.tile([128, BC], F32, tag="mmps", bufs=4)
            for kc in range(KH):
                nc.tensor.matmul(ps[:], W_ms2[:, kc, sl], qmT_bf[:, kc, :],
                                 start=(kc == 0), stop=False)
            nc.tensor.matmul(ps[:], b_ms2r[0:1, sl], ones_row[0:1, 0:BC],
                             start=False, stop=True)
            nc.vector.tensor_add(qT_bf[:, mc, :], ps[:], qnT_bf[:, mc, :])

        # qw = s4 * (q2 @ f1_W)  (token-major bf16, prescaled to match fp8 psum)
        qw_bf = qp.tile([BC, D], BF16)
        for ng in range(2):
            sl = slice(ng * 512, (ng + 1) * 512)
            W_f1h = qp.tile([128, KC, 512], BF16, tag="wf1h", bufs=1)
            nc.scalar.dma_start(W_f1h[:], f1W_r[:, :, sl])
            ps = psum.tile([BC, 512], F32, tag="mmps", bufs=4)
            for kc in range(KC):
                nc.tensor.matmul(ps[:], qT_bf[:, kc, :], W_f1h[:, kc, :],
                                 start=(kc == 0), stop=(kc == KC - 1))
            nc.scalar.activation(qw_bf[:, sl], ps[:], AF.Identity, scale=float(s4))
        nc.sync.dma_start(qw_scr[:, :], qw_bf[:])

        # fill vector for masked tokens: relu(f1_b)
        fb_row = qp.tile([1, D], F32)
        nc.gpsimd.dma_start(fb_row[:], wi["f1_b"])
        fb_out = qp.tile([1, D], F32)
        nc.vector.tensor_scalar_max(fb_out[:], fb_row[:], 0.0)
        nc.sync.dma_start(fillv[:, :], fb_out[:])

    for j in range(len(tiles)):
        emit_C(j)
        if j + 3 < len(tiles):
            emit_A(j + 3)
        emit_B(j)


def _build(plan, scales):
    nc = bacc.Bacc("TRN2", target_bir_lowering=False, debug=False)
    ctx = ExitStack()
    with tile.TileContext(nc) as tc, ctx:
        _emit(ctx, tc, plan, scales)
    nc.compile()
    return nc


_NC_CACHE = {}


def _get_nc(plan_key, plan, scales):
    if plan_key not in _NC_CACHE:
        _NC_CACHE[plan_key] = _build(plan, scales)
    return _NC_CACHE[plan_key]


def _build_ind_blob(tiles):
    sz = sum(t["nb"] * t["nt"] for t in tiles)
    blob = np.zeros(sz, dtype=NPBF16)
    off = 0
    for t in tiles:
        ind = np.zeros((t["nb"], t["nt"]), dtype=NPBF16)
        for row, lo, hi in t["segs"]:
            ind[row, lo:hi] = 1
        blob[off:off + ind.size] = ind.ravel()
        off += ind.size
    return blob


def _run_cores(ncs, in_maps, trace=False):
    """Dispatch one compiled program per core, concurrently."""
    import jax
    from concourse import bass2jax
    from concourse.bass2jax import _bass_exec_p, install_neuronx_cc_hook

    install_neuronx_cc_hook()
    devices = jax.devices()[:NCORES]

    def make_jit(nc):
        in_names, out_names, out_avals, zero_outs = [], [], [], []
        for alloc in nc.m.functions[0].allocations:
            if not isinstance(alloc, mybir.MemoryLocationSet):
                continue
            name = alloc.memorylocations[0].name
            if alloc.kind == "ExternalInput":
                in_names.append(name)
            elif alloc.kind == "ExternalOutput":
                out_names.append(name)
                shape = tuple(alloc.tensor_shape)
                dtype = mybir.dt.np(alloc.dtype)
                out_avals.append(jax.core.ShapedArray(shape, dtype))
                zero_outs.append(np.zeros(shape, dtype))
        n_params = len(in_names)
        all_names = in_names + out_names

        def _body(*args):
            outs = _bass_exec_p.bind(
                *args,
                out_avals=tuple(out_avals),
                in_names=tuple(all_names),
                out_names=tuple(out_names),
                lowering_input_output_aliases=(),
                sim_require_finite=True,
                sim_require_nnan=True,
                nc=nc,
            )
            return tuple(outs)

        donate = tuple(range(n_params, n_params + len(out_names)))
        return (jax.jit(_body, donate_argnums=donate, keep_unused=True),
                in_names, out_names, zero_outs)

    with ThreadPoolExecutor(NCORES) as ex:
        jits = list(ex.map(make_jit, ncs))

    def launch(c):
        jitted, in_names, out_names, zero_outs = jits[c]
        vals = dict(in_maps[c])
        pid = ncs[c].partition_id_tensor
        if pid is not None:
            vals[pid.name] = np.array([[c]], dtype=np.uint32)
        args = [jax.device_put(np.asarray(vals[n]), devices[c]) for n in in_names]
        zz = [jax.device_put(z, devices[c]) for z in zero_outs]
        outs = jitted(*args, *zz)
        return dict(zip(out_names, outs))

    def run_all():
        with ThreadPoolExecutor(NCORES) as ex:
            outs = list(ex.map(launch, range(NCORES)))
        return [{k: np.asarray(v) for k, v in o.items()} for o in outs]

    global LAST_EXEC_NS, _LAST_TRACE
    if trace:
        import glob as globmod
        import tempfile
        from antenv.axon_hooks import get_axon_ntff_profile_hook
        hook = get_axon_ntff_profile_hook()
        neff_dir = tempfile.mkdtemp()
        if hook is None:
            results = run_all()
        else:
            run_all()  # warm: jit trace + NEFF compile before the profiled run
            with hook(neff_dir, [0]):
                results = run_all()
            try:
                import re
                import shutil
                import gauge.profiler
                from concourse._compat import FishPath
                ntffs = sorted(globmod.glob(os.path.join(neff_dir, "*_body*.ntff")))
                times = []
                insts_best = None
                for ntff in ntffs:
                    m = re.search(r"executable(\d+)", os.path.basename(ntff))
                    exe = m.group(1)
                    sub = os.path.join(neff_dir, f"exe{exe}")
                    os.makedirs(sub, exist_ok=True)
                    for fpath in globmod.glob(os.path.join(neff_dir, f"*executable{exe}*")):
                        if os.path.isfile(fpath):
                            shutil.copy(fpath, sub)
                    profile = gauge.profiler.Profile(
                        profile_path=FishPath(sub), kernel_dev_mode=True,
                        profile_on_exit=False, bass_kernel=ncs[0].m,
                        offline_processing=True, fname="*_body*",
                        metadata={"artifacts_path": sub})
                    pr = profile.to_perfetto(model_index=(0,))
                    if pr:
                        times.append(pr[0].exec_time_ns)
                        if pr[0].exec_time_ns == max(times):
                            insts_best = (pr[0].insts, pr[0].trace_path)
                if times:
                    LAST_EXEC_NS = max(times)
                    _LAST_TRACE = insts_best
                    print(f"per-core exec ns: {sorted(times)}", file=sys.stderr)
            except Exception as e:
                print(f"profile post-processing failed: {e!r}", file=sys.stderr)
    else:
        results = run_all()
    return results


def _arrange_w(W, np_dt):
    # [D, m] -> [128, KC*m] matching "(kc p) m -> p kc m" on device
    Dd, m = W.shape
    kchunks = Dd // 128
    return np.ascontiguousarray(
        np.asarray(W).reshape(kchunks, 128, m).transpose(1, 0, 2).reshape(128, kchunks * m).astype(np_dt))


def _wscale(W):
    m = float(np.abs(W).max())
    if m <= 0:
        return 1.0
    return float(2.0 ** np.floor(np.log2(200.0 / m)))


def _fp8q(W, s):
    return np.ascontiguousarray(
        np.clip(np.asarray(W, np.float32) * s, -240.0, 240.0).astype(NPFP8))


def kernel(rgns, Unet_segs, region_lens, mi_W1, mi_b1, mi_W2, mi_b2,
           ms_W1, ms_b1, ms_W2, ms_b2, seg_W, seg_b, ln_g, ln_b,
           sc_W, sc_b, f1_W, f1_b):
    _wire_ntff_hook()

    f = lambda a: np.ascontiguousarray(np.asarray(a, dtype=np.float32))
    bf = lambda a: np.ascontiguousarray(np.asarray(a, dtype=np.float32).astype(NPBF16))
    rgns = f(rgns)
    unet = np.asarray(Unet_segs, np.float32).reshape(B, SEG_C, 49).astype(NPBF16)
    lens = np.clip(np.asarray(region_lens).astype(np.int64), 0, R)

    s1, s2, s3, s4 = (_wscale(mi_W1), _wscale(mi_W2), _wscale(sc_W), _wscale(f1_W))
    weights = {
        "mi_W1q": _arrange_w(_fp8q(mi_W1, s1), NPFP8), "mi_b1": f(mi_b1).reshape(1, H),
        "mi_W2q": _arrange_w(_fp8q(mi_W2, s2), NPFP8), "mi_b2": f(mi_b2).reshape(1, D),
        "ms_W1": _arrange_w(np.asarray(ms_W1, np.float32), NPBF16), "ms_b1": f(ms_b1).reshape(1, H),
        "ms_W2": _arrange_w(np.asarray(ms_W2, np.float32), NPBF16), "ms_b2": f(ms_b2).reshape(1, D),
        "seg_W": bf(seg_W), "seg_b": f(seg_b).reshape(1, D),
        "ln_g": f(ln_g).reshape(1, D), "ln_b": f(ln_b).reshape(1, D),
        "sc_Wq": _arrange_w(_fp8q(sc_W, s3), NPFP8), "sc_b": f(sc_b).reshape(1, D),
        "f1_Wq": _arrange_w(_fp8q(f1_W, s4), NPFP8), "f1_W": bf(f1_W), "f1_b": f(f1_b).reshape(1, D),
    }

    # balanced batch assignment: 128 batches per core, equalize token counts
    order = np.argsort(-lens, kind="stable")
    loads = np.zeros(NCORES, dtype=np.int64)
    counts = np.zeros(NCORES, dtype=np.int64)
    assign = [[] for _ in range(NCORES)]
    for b in order:
        open_cores = [c for c in range(NCORES) if counts[c] < BC]
        c = min(open_cores, key=lambda c: loads[c])
        assign[c].append(int(b))
        loads[c] += int(lens[b])
        counts[c] += 1
    batches = [np.sort(np.array(a, dtype=np.int64)) for a in assign]

    rflat = rgns.reshape(B * R, D)
    in_maps, plans, vrows = [], [], []
    for c in range(NCORES):
        bl = batches[c]
        lens_c = lens[bl]
        plan = _make_plan(lens_c)
        plans.append(plan)
        rows = np.concatenate([bl[i] * R + np.arange(lens_c[i]) for i in range(BC)])
        vrows.append(rows)
        xc = rflat[rows]
        x8b_parts, xbb_parts = [], []
        for t in plan[0]:
            blk = xc[t["t0"]:t["t0"] + t["nt"], :].T  # [D, nt]
            blk = blk.reshape(KC, 128, t["nt"]).transpose(1, 0, 2).reshape(128, -1)
            x8b_parts.append(np.clip(blk, -240.0, 240.0).astype(NPFP8))
            xbb_parts.append(blk.astype(NPBF16))
        in_maps.append(dict(
            x8B=np.ascontiguousarray(np.concatenate(x8b_parts, axis=1)),
            xbB=np.ascontiguousarray(np.concatenate(xbb_parts, axis=1)),
            unet=np.ascontiguousarray(unet[bl]),
            ind=_build_ind_blob(plan[0]),
            **weights,
        ))

    def plan_key(c):
        return (tuple((t["t0"], t["nt"], t["b0"], t["nb"], tuple(t["segs"]))
                      for t in plans[c][0]), (s1, s2, s3, s4))

    keys = [plan_key(c) for c in range(NCORES)]
    uniq = {}
    for c in range(NCORES):
        if keys[c] not in uniq:
            uniq[keys[c]] = None
    with ThreadPoolExecutor(min(8, len(uniq))) as ex:
        built = dict(zip(uniq.keys(),
                         ex.map(lambda k: _get_nc(k, plans[keys.index(k)], (s1, s2, s3, s4)),
                                list(uniq.keys()))))
    ncs = [built[keys[c]] for c in range(NCORES)]

    trace = bool(int(os.environ.get("BASSK_TRACE", "0")))
    results = _run_cores(ncs, in_maps, trace=trace)

    out = np.empty((B * R, D), np.float32)
    out[:] = results[0]["fillv"].reshape(1, D)
    for c in range(NCORES):
        ob = results[c]["outB"]
        bo = 0
        for t in plans[c][0]:
            nt = t["nt"]
            blk = ob[:, bo:bo + KC * nt].reshape(128, KC, nt)
            blk = blk.transpose(1, 0, 2).reshape(D, nt).T.astype(np.float32)
            out[vrows[c][t["t0"]:t["t0"] + nt]] = blk
            bo += KC * nt
    return out.reshape(B, R, D)
